# revision 2
# baseline (speedup 1.0000x reference)
"""Complex transformer block (LN->attn->LN->MLP, complex arithmetic) on 8 TRN2 cores.

Sharding: core c handles (batch b = c//2, sequence half = c%2). Weights are
shipped ONCE (sharded 1/8 per core, bf16) and redistributed on-device with an
AllGather; activations/outputs travel as bf16. Each core computes K/V over the
full 1024-token sequence of its batch (the only duplicated work) and
queries/MLP over its own 512 tokens.

Layout: activations are feature-major [feature partition-blocks, tokens].
Complex tensors are realified as separate real/imag feature planes. LayerNorm
is fused into the following matmul: per-token stats (mu_r, mu_i, std) are
appended as 3 extra contraction rows with matching weight columns, and the
per-token rstd is applied by the PSUM-eviction multiply. Attention scores are
computed transposed ([t2, t1]) so softmax sums reduce via ones-matmuls, and V
is produced pre-transposed by swapping matmul operands. All matmuls run in
bf16 at full PE rate.
"""
import sys
sys.path.insert(0, "/opt/trn_rl_repo")

from contextlib import ExitStack

import numpy as np
import ml_dtypes

import concourse.bacc as bacc
import concourse.bass as bass
import concourse.mybir as mybir
import concourse.tile as tile
from concourse.bass_utils import run_bass_kernel_spmd

# Prefer the table set that covers the whole softmax chain (square+ln+exp)
# so the greedy act-table-load pass doesn't thrash sets on every block.
_orig_get_tables = bacc.get_activation_tables


def _reordered_tables(arch):
    t = _orig_get_tables(arch)
    keep = {"natural_log_exp_and_others", "gelu_and_others"}
    return {k: (v if k in keep else set()) for k, v in t.items()}


bacc.get_activation_tables = _reordered_tables

dt = mybir.dt
AF = mybir.ActivationFunctionType
ALU = mybir.AluOpType
BF16 = ml_dtypes.bfloat16

B, N, C, H, DH, HID = 4, 1024, 768, 12, 64, 3072
NCORES = 8
OWN = 512          # tokens per core
SCALE = DH ** -0.5
EPS = 1e-5
MC = 256           # MLP token chunk

# gathered (full) weight shapes, all bf16; shards are flat 1/8 slices
W_SHAPES = {
    "w_qkv": (H, 12, 128, 384),
    "w_v": (12, 128, 1536),
    "w_proj": (12, 12, 128, 128),
    "w_fc1r": (24, 6, 128, 128),
    "w_fc1i": (24, 6, 128, 128),
    "w_fc2r": (6, 24, 128, 128),
    "w_fc2i": (6, 24, 128, 128),
}


def _shard_cols(name):
    n = int(np.prod(W_SHAPES[name]))
    assert n % (8 * 128) == 0
    return n // (8 * 128)


# --------------------------------------------------------------------------
# device program
# --------------------------------------------------------------------------

def build_nc():
    nc = bacc.Bacc(trn_type="TRN2", target_bir_lowering=False, num_devices=8)
    f32 = dt.float32
    bf16 = dt.bfloat16

    # ---- DRAM I/O ----
    x_r = nc.dram_tensor("x_r", [12, 128, N], bf16, kind="ExternalInput")
    shards = {}
    for nm in W_SHAPES:
        shards[nm] = nc.dram_tensor(f"sh_{nm}", [128, _shard_cols(nm)], bf16,
                                    kind="ExternalInput")
    w_qkv_s = nc.dram_tensor("w_qkv_s", [H, 4, 384], bf16, kind="ExternalInput")
    w_v_s = nc.dram_tensor("w_v_s", [4, 1536], bf16, kind="ExternalInput")
    w_proj_s = nc.dram_tensor("w_proj_s", [12, 4, 128], bf16, kind="ExternalInput")
    w_fc1_s = nc.dram_tensor("w_fc1_s", [24, 4, 256], bf16, kind="ExternalInput")
    w_fc2_s = nc.dram_tensor("w_fc2_s", [6, 4, 256], bf16, kind="ExternalInput")
    ones_col = nc.dram_tensor("ones_col", [128, 1], bf16, kind="ExternalInput")
    ones_ab = nc.dram_tensor("ones_ab", [128, 4], bf16, kind="ExternalInput")
    ones_s = nc.dram_tensor("ones_s", [128, 1], bf16, kind="ExternalInput")
    stat_one = nc.dram_tensor("stat_one", [4, OWN], bf16, kind="ExternalInput")

    out_fm = nc.dram_tensor("out_fm", [12, 128, OWN], bf16, kind="ExternalOutput")

    # gathered full weights (Shared scratchpad, written by AllGather)
    gath = {}
    for nm, shp in W_SHAPES.items():
        gath[nm] = nc.dram_tensor(f"g_{nm}", list(shp), bf16,
                                  kind="Internal", addr_space="Shared")
    w_qkv = gath["w_qkv"]
    w_v = gath["w_v"]
    w_proj = gath["w_proj"]
    w_fc1r = gath["w_fc1r"]
    w_fc1i = gath["w_fc1i"]
    w_fc2r = gath["w_fc2r"]
    w_fc2i = gath["w_fc2i"]

    with tile.TileContext(nc) as tc, ExitStack() as top:
        # ---- weight redistribution: bounce + AllGather, in order of use ----
        gpool = top.enter_context(tc.tile_pool(name="gpool", bufs=1, space="DRAM"))
        for nm in ["w_qkv", "w_v", "w_proj", "w_fc1r", "w_fc1i",
                   "w_fc2r", "w_fc2i"]:
            bounce = gpool.tile([128, _shard_cols(nm)], bf16, name=f"bn_{nm}")
            nc.sync.dma_start(bounce[:], shards[nm][:])
            nc.gpsimd.collective_compute(
                "AllGather",
                ALU.bypass,
                replica_groups=[list(range(8))],
                ins=[bounce[:].opt()],
                outs=[gath[nm][:].opt()],
            )

        consts = top.enter_context(tc.tile_pool(name="consts", bufs=1))
        t_ones_col = consts.tile([128, 1], bf16)
        t_ones_ab = consts.tile([128, 4], bf16)
        t_ones_s = consts.tile([128, 1], bf16)
        t_stat_one = consts.tile([4, OWN], bf16)
        t_eps = consts.tile([1, 1], f32)
        nc.sync.dma_start(t_ones_col[:], ones_col[:])
        nc.sync.dma_start(t_ones_ab[:], ones_ab[:])
        nc.sync.dma_start(t_ones_s[:], ones_s[:])
        nc.sync.dma_start(t_stat_one[:], stat_one[:])
        nc.vector.memset(t_eps[:], EPS)

        poolR1 = top.enter_context(tc.tile_pool(name="poolR1", bufs=1))
        xr1 = poolR1.tile([128, 12, OWN], f32, name="xr1")

        with ExitStack() as es_x:
            poolX = es_x.enter_context(tc.tile_pool(name="poolX", bufs=1))
            xr = poolX.tile([128, 12, N], bf16, name="xr")
            pdram = es_x.enter_context(
                tc.tile_pool(name="pdram", bufs=1, space="DRAM"))
            rstd_dram = pdram.tile([1, N], f32, name="rstd_dram")
            stat1 = poolX.tile([4, N], bf16, name="stat1")
            rstd_bc1 = poolX.tile([128, N], f32, name="rstd_bc1")
            rstdT = poolX.tile([128, 8], f32, name="rstdT")
            for kb in range(12):
                nc.sync.dma_start(xr[:, kb, :], x_r[kb])
            # residual copy (bf16 -> f32)
            for kb in range(12):
                nc.vector.tensor_copy(xr1[:, kb, :], xr[:, kb, 0:OWN])

            # ---------------- phase A: LN1 stats over full sequence --------
            with ExitStack() as es_a:
                pa = es_a.enter_context(tc.tile_pool(name="pa_sb", bufs=12))
                pa_ps = es_a.enter_context(
                    tc.tile_pool(name="pa_ps", bufs=2, space="PSUM"))
                pa_sc = es_a.enter_context(tc.tile_pool(name="pa_sc", bufs=2))
                sqs = []
                for kb in range(12):
                    sq = pa.tile([128, N], bf16, tag="sq", name=f"sq{kb}")
                    nc.scalar.activation(sq[:], xr[:, kb, :], AF.Square)
                    sqs.append(sq)
                for ch in range(2):
                    sl = slice(ch * 512, ch * 512 + 512)
                    mu_ps = pa_ps.tile([2, 512], f32, tag="mu", name=f"mu{ch}")
                    s_ps = pa_ps.tile([1, 512], f32, tag="s", name=f"s{ch}")
                    for kb in range(12):
                        lhs = t_ones_ab[:, 0:2] if kb < 6 else t_ones_ab[:, 2:4]
                        nc.tensor.matmul(mu_ps[:], lhs, xr[:, kb, sl],
                                         start=(kb == 0), stop=(kb == 11))
                        nc.tensor.matmul(s_ps[:], t_ones_s[:], sqs[kb][:, sl],
                                         start=(kb == 0), stop=(kb == 11))
                    # var = S - mu_r^2 - mu_i^2 ; std = exp(.5 ln(var+eps))
                    mu_sb = pa_sc.tile([2, 512], f32, tag="musb", name=f"musb{ch}")
                    mu_fl = pa_sc.tile([1, 2, 512], f32, tag="mufl", name=f"mufl{ch}")
                    var = pa_sc.tile([1, 512], f32, tag="var", name=f"var{ch}")
                    lnv = pa_sc.tile([1, 512], f32, tag="lnv", name=f"lnv{ch}")
                    nc.vector.tensor_copy(mu_sb[:], mu_ps[:])
                    nc.sync.dma_start(mu_fl[:, 0, :], mu_sb[0:1, :])
                    nc.sync.dma_start(mu_fl[:, 1, :], mu_sb[1:2, :])
                    sq_mu = pa_sc.tile([1, 2, 512], f32, tag="sqmu", name=f"sqmu{ch}")
                    nc.vector.tensor_tensor(sq_mu[:], mu_fl[:], mu_fl[:],
                                            op=ALU.mult)
                    nc.vector.tensor_tensor(var[:], s_ps[:], sq_mu[:, 0, :],
                                            op=ALU.subtract)
                    nc.vector.tensor_tensor(var[:], var[:], sq_mu[:, 1, :],
                                            op=ALU.subtract)
                    nc.scalar.activation(lnv[:], var[:], AF.Ln, bias=t_eps[:])
                    # stats rows: 0=mu_r 1=mu_i 2=std
                    nc.vector.tensor_copy(stat1[0:2, sl], mu_sb[:])
                    std_row = pa_sc.tile([1, 512], bf16, tag="stdr", name=f"stdr{ch}")
                    nc.scalar.activation(std_row[:], lnv[:], AF.Exp, scale=0.5)
                    nc.sync.dma_start(stat1[2:3, sl], std_row[:])
                    rstd_row = pa_sc.tile([1, 512], f32, tag="rst", name=f"rst{ch}")
                    nc.scalar.activation(rstd_row[:], lnv[:], AF.Exp, scale=-0.5)
                    nc.sync.dma_start(rstd_dram[:, sl], rstd_row[:])
                    bcast = bass.AP(tensor=rstd_dram.tensor,
                                    offset=rstd_dram[:, sl].offset,
                                    ap=[[0, 128]] + rstd_dram[:, sl].ap[1:])
                    nc.sync.dma_start(rstd_bc1[:, sl], bcast)
                # rstd transposed: rstdT[p, t2b] = rstd[t2b*128 + p]
                nc.sync.dma_start(
                    rstdT[:],
                    rstd_dram[:].rearrange("o (b p) -> (o p) b", p=128))

            # ---------------- phase BC: qkv + attention per head ----------
            es_attn = ExitStack()
            attnp = es_attn.enter_context(tc.tile_pool(name="attnp", bufs=1))
            attn = attnp.tile([128, 12, OWN], bf16, name="attn")
            es_b = ExitStack()
            pq = es_b.enter_context(tc.tile_pool(name="pq", bufs=1))
            pk = es_b.enter_context(tc.tile_pool(name="pk", bufs=1))
            pvt = es_b.enter_context(tc.tile_pool(name="pvt", bufs=2))
            pwv = es_b.enter_context(tc.tile_pool(name="pwv", bufs=1))
            pwq = es_b.enter_context(tc.tile_pool(name="pwq", bufs=2))
            pet = es_b.enter_context(tc.tile_pool(name="pet", bufs=6))
            psc = es_b.enter_context(tc.tile_pool(name="psc", bufs=6))
            prd = es_b.enter_context(tc.tile_pool(name="prd", bufs=2))
            ps_rot = es_b.enter_context(
                tc.tile_pool(name="ps_rot", bufs=6, space="PSUM"))
            ps_acc = es_b.enter_context(
                tc.tile_pool(name="ps_acc", bufs=2, space="PSUM"))
            pdram_rd = es_b.enter_context(
                tc.tile_pool(name="pdram_rd", bufs=2, space="DRAM"))
            vt_pair = None
            et_fifo = []
            acc_ps = {}
            LAG = 4

            def emit_avden(ent):
                h2, t2b2, et2, vt2 = ent
                slot2 = h2 % 2
                if t2b2 == 0:
                    acc_ps[h2] = (
                        ps_acc.tile([128, OWN], f32, tag="av", name=f"av{h2}",
                                    bufs=1),
                        ps_acc.tile([1, OWN], f32, tag="den", name=f"den{h2}",
                                    bufs=1),
                    )
                av2, den2 = acc_ps[h2]
                nc.tensor.matmul(den2[:], t_ones_col[:], et2[:],
                                 start=(t2b2 == 0), stop=(t2b2 == 7))
                dsl2 = slice(slot2 * 128, slot2 * 128 + 128)
                nc.tensor.matmul(av2[:], vt2[:, t2b2, dsl2], et2[:],
                                 start=(t2b2 == 0), stop=(t2b2 == 7))
                if t2b2 == 7:
                    den_sb = prd.tile([1, OWN], f32, tag="den_sb",
                                      name=f"dsb{h2}", bufs=1)
                    rd = prd.tile([1, OWN], f32, tag="rd", name=f"rd{h2}",
                                  bufs=1)
                    nc.vector.tensor_copy(den_sb[:], den2[:])
                    nc.vector.reciprocal(rd[:], den_sb[:])
                    rd_dram = pdram_rd.tile([1, OWN], f32, tag="rdd",
                                            name=f"rdd{h2}")
                    nc.sync.dma_start(rd_dram[:], rd[:])
                    rd_bc = prd.tile([128, OWN], f32, tag="rd_bc",
                                     name=f"rdbc{h2}", bufs=1)
                    rd_bcast_ap = bass.AP(tensor=rd_dram.tensor,
                                          offset=rd_dram[:].offset,
                                          ap=[[0, 128]] + rd_dram[:].ap[1:])
                    nc.sync.dma_start(rd_bc[:], rd_bcast_ap)
                    nc.vector.tensor_tensor(attn[:, h2, :], av2[:], rd_bc[:],
                                            op=ALU.mult)
                    del acc_ps[h2]

            for h in range(H):
                pair, slot = divmod(h, 2)
                # qkv for head h: q1=[q_r;-q_i], q3=[q_i;q_r], k=[k_r;k_i]
                q_t = pq.tile([128, 2, OWN], bf16, tag="q", name=f"q{h}")
                k_t = pk.tile([128, N], bf16, tag="k", name=f"k{h}")
                wqs_t = pwq.tile([4, 384], bf16, tag="wqs", name=f"wqs{h}")
                nc.sync.dma_start(wqs_t[:], w_qkv_s[h])
                q1_ps = ps_rot.tile([128, OWN], f32, tag="rot", name=f"q1ps{h}")
                q3_ps = ps_rot.tile([128, OWN], f32, tag="rot", name=f"q3ps{h}")
                for kb in range(12):
                    wq_t = pwq.tile([128, 256], bf16, tag="wq",
                                    name=f"wq{h}_{kb}")
                    nc.sync.dma_start(wq_t[:], w_qkv[h, kb, :, 0:256])
                    st = (kb == 0)
                    nc.tensor.matmul(q1_ps[:], wq_t[:, 0:128],
                                     xr[:, kb, 0:OWN], start=st, stop=False)
                    nc.tensor.matmul(q3_ps[:], wq_t[:, 128:256],
                                     xr[:, kb, 0:OWN], start=st, stop=False)
                nc.tensor.matmul(q1_ps[:], wqs_t[:, 0:128], stat1[:, 0:OWN],
                                 start=False, stop=True)
                nc.tensor.matmul(q3_ps[:], wqs_t[:, 128:256], stat1[:, 0:OWN],
                                 start=False, stop=True)
                nc.vector.tensor_tensor(q_t[:, 0, :], q1_ps[:],
                                        rstd_bc1[:, 0:OWN], op=ALU.mult)
                nc.vector.tensor_tensor(q_t[:, 1, :], q3_ps[:],
                                        rstd_bc1[:, 0:OWN], op=ALU.mult)
                k0_ps = ps_rot.tile([128, 512], f32, tag="rot", name=f"k0ps{h}")
                k1_ps = ps_rot.tile([128, 512], f32, tag="rot", name=f"k1ps{h}")
                for kb in range(12):
                    wk_t = pwq.tile([128, 128], bf16, tag="wk",
                                    name=f"wk{h}_{kb}")
                    nc.sync.dma_start(wk_t[:], w_qkv[h, kb, :, 256:384])
                    st = (kb == 0)
                    nc.tensor.matmul(k0_ps[:], wk_t[:],
                                     xr[:, kb, 0:512], start=st, stop=False)
                    nc.tensor.matmul(k1_ps[:], wk_t[:],
                                     xr[:, kb, 512:N], start=st, stop=False)
                nc.tensor.matmul(k0_ps[:], wqs_t[:, 256:384], stat1[:, 0:512],
                                 start=False, stop=True)
                nc.tensor.matmul(k1_ps[:], wqs_t[:, 256:384], stat1[:, 512:N],
                                 start=False, stop=True)
                nc.vector.tensor_tensor(k_t[:, 0:512], k0_ps[:],
                                        rstd_bc1[:, 0:512], op=ALU.mult)
                nc.vector.tensor_tensor(k_t[:, 512:N], k1_ps[:],
                                        rstd_bc1[:, 512:N], op=ALU.mult)
                if slot == 0:
                    # V^T for this head pair: [t2, d] via swapped operands
                    wv_t = pwv.tile([128, 12, 256], bf16, tag="wv",
                                    name=f"wv{pair}")
                    wv_s = pwv.tile([4, 256], bf16, tag="wvs",
                                    name=f"wvs{pair}")
                    csl = slice(pair * 256, pair * 256 + 256)
                    nc.sync.dma_start(wv_t[:], w_v[:, :, csl]
                                      .rearrange("b p n -> p b n"))
                    nc.sync.dma_start(wv_s[:], w_v_s[:, csl])
                    vt_pair = pvt.tile([128, 8, 256], bf16, tag="vt",
                                       name=f"vt{pair}")
                    for t2b in range(8):
                        t2s = slice(t2b * 128, t2b * 128 + 128)
                        vt_ps = ps_rot.tile([128, 256], f32, tag="rot",
                                            name=f"vtps{pair}_{t2b}")
                        for kb in range(12):
                            nc.tensor.matmul(vt_ps[:], xr[:, kb, t2s],
                                             wv_t[:, kb, :],
                                             start=(kb == 0), stop=False)
                        nc.tensor.matmul(vt_ps[:], stat1[:, t2s], wv_s[:],
                                         start=False, stop=True)
                        nc.vector.tensor_scalar(
                            vt_pair[:, t2b, :], vt_ps[:],
                            rstdT[:, t2b:t2b + 1], None, op0=ALU.mult)
                # scores + exp chain; den/av matmuls lag by LAG blocks
                for t2b in range(8):
                    t2s = slice(t2b * 128, t2b * 128 + 128)
                    sr_ps = ps_rot.tile([128, OWN], f32, tag="rot",
                                        name=f"sr{h}_{t2b}")
                    si_ps = ps_rot.tile([128, OWN], f32, tag="rot",
                                        name=f"si{h}_{t2b}")
                    nc.tensor.matmul(sr_ps[:], k_t[:, t2s], q_t[:, 0, :],
                                     start=True, stop=True)
                    nc.tensor.matmul(si_ps[:], k_t[:, t2s], q_t[:, 1, :],
                                     start=True, stop=True)
                    sqr = psc.tile([128, OWN], f32, tag="sc",
                                   name=f"sqr{h}_{t2b}")
                    sqi = psc.tile([128, OWN], f32, tag="sc",
                                   name=f"sqi{h}_{t2b}")
                    nc.scalar.activation(sqr[:], sr_ps[:], AF.Square)
                    nc.scalar.activation(sqi[:], si_ps[:], AF.Square)
                    # in-place chain on sqr: m2 -> ln -> 0.5ln -> mag
                    nc.vector.tensor_tensor(sqr[:], sqr[:], sqi[:], op=ALU.add)
                    nc.scalar.activation(sqr[:], sqr[:], AF.Ln)
                    nc.scalar.activation(sqr[:], sqr[:], AF.Exp, scale=0.5)
                    et = pet.tile([128, OWN], bf16, tag="et",
                                  name=f"et{h}_{t2b}")
                    nc.scalar.activation(et[:], sqr[:], AF.Exp)
                    et_fifo.append((h, t2b, et, vt_pair))
                    while len(et_fifo) > LAG:
                        emit_avden(et_fifo.pop(0))
            for ent in et_fifo:
                emit_avden(ent)
            et_fifo.clear()
            es_b.close()

            # ------------- phase D: proj + residual --------------------
            r1r = poolR1.tile([128, 12, OWN], bf16, name="r1r")
            with ExitStack() as es_d:
                pwp = es_d.enter_context(tc.tile_pool(name="pwp", bufs=3))
                ps_d = es_d.enter_context(
                    tc.tile_pool(name="ps_d", bufs=4, space="PSUM"))
                for opb in range(12):
                    wp_t = pwp.tile([128, 12, 128], bf16, tag="wp",
                                    name=f"wp{opb}")
                    wps_t = pwp.tile([4, 128], bf16, tag="wps",
                                     name=f"wps{opb}")
                    nc.sync.dma_start(wp_t[:], w_proj[opb]
                                      .rearrange("b p n -> p b n"))
                    nc.sync.dma_start(wps_t[:], w_proj_s[opb])
                    pr_ps = ps_d.tile([128, OWN], f32, tag="pr",
                                      name=f"prps{opb}")
                    for kb in range(12):
                        nc.tensor.matmul(pr_ps[:], wp_t[:, kb, :],
                                         attn[:, kb, :],
                                         start=(kb == 0), stop=False)
                    nc.tensor.matmul(pr_ps[:], wps_t[:], t_stat_one[:],
                                     start=False, stop=True)
                    nc.vector.tensor_tensor(xr1[:, opb, :], pr_ps[:],
                                            xr1[:, opb, :], op=ALU.add)
                    nc.vector.tensor_copy(r1r[:, opb, :], xr1[:, opb, :])
            es_attn.close()

        # ---------------- phase E: LN2 stats over own tokens --------------
        stat2 = poolR1.tile([4, OWN], bf16, name="stat2")
        rstd2_bc = poolR1.tile([128, OWN], f32, name="rstd2_bc")
        with ExitStack() as es_e:
            pe = es_e.enter_context(tc.tile_pool(name="pe_sb", bufs=1))
            pdram2 = es_e.enter_context(
                tc.tile_pool(name="pdram2", bufs=1, space="DRAM"))
            pe_ps = es_e.enter_context(
                tc.tile_pool(name="pe_ps", bufs=2, space="PSUM"))
            sq2s = []
            for kb in range(12):
                sq2 = pe.tile([128, OWN], bf16, tag="sq2", name=f"sq2_{kb}", bufs=12)
                nc.scalar.activation(sq2[:], r1r[:, kb, :], AF.Square)
                sq2s.append(sq2)
            mu2_ps = pe_ps.tile([2, OWN], f32, tag="mu2", name="mu2")
            s2_ps = pe_ps.tile([1, OWN], f32, tag="s2", name="s2")
            for kb in range(12):
                lhs = t_ones_ab[:, 0:2] if kb < 6 else t_ones_ab[:, 2:4]
                nc.tensor.matmul(mu2_ps[:], lhs, r1r[:, kb, :],
                                 start=(kb == 0), stop=(kb == 11))
                nc.tensor.matmul(s2_ps[:], t_ones_s[:], sq2s[kb][:],
                                 start=(kb == 0), stop=(kb == 11))
            mu2_sb = pe.tile([2, OWN], f32, tag="emusb", name="emusb")
            mu2_fl = pe.tile([1, 2, OWN], f32, tag="emufl", name="emufl")
            var = pe.tile([1, OWN], f32, tag="evar", name="evar")
            lnv = pe.tile([1, OWN], f32, tag="elnv", name="elnv")
            nc.vector.tensor_copy(mu2_sb[:], mu2_ps[:])
            nc.sync.dma_start(mu2_fl[:, 0, :], mu2_sb[0:1, :])
            nc.sync.dma_start(mu2_fl[:, 1, :], mu2_sb[1:2, :])
            sq_mu2 = pe.tile([1, 2, OWN], f32, tag="esqmu", name="esqmu")
            nc.vector.tensor_tensor(sq_mu2[:], mu2_fl[:], mu2_fl[:], op=ALU.mult)
            nc.vector.tensor_tensor(var[:], s2_ps[:], sq_mu2[:, 0, :],
                                    op=ALU.subtract)
            nc.vector.tensor_tensor(var[:], var[:], sq_mu2[:, 1, :],
                                    op=ALU.subtract)
            nc.scalar.activation(lnv[:], var[:], AF.Ln, bias=t_eps[:])
            nc.vector.tensor_copy(stat2[0:2, :], mu2_sb[:])
            std2_row = pe.tile([1, OWN], bf16, tag="estd", name="estd")
            nc.scalar.activation(std2_row[:], lnv[:], AF.Exp, scale=0.5)
            nc.sync.dma_start(stat2[2:3, :], std2_row[:])
            rstd2_row = pe.tile([1, OWN], f32, tag="ers", name="ers")
            nc.scalar.activation(rstd2_row[:], lnv[:], AF.Exp, scale=-0.5)
            rstd2_dram = pdram2.tile([1, OWN], f32, name="rstd2_dram")
            nc.sync.dma_start(rstd2_dram[:], rstd2_row[:])
            bcast2 = bass.AP(tensor=rstd2_dram.tensor, offset=rstd2_dram[:].offset,
                             ap=[[0, 128]] + rstd2_dram[:].ap[1:])
            nc.sync.dma_start(rstd2_bc[:], bcast2)

        # ---------------- phase F: MLP per 256-token chunk ----------------
        with ExitStack() as es_f:
            pneg = es_f.enter_context(tc.tile_pool(name="pneg", bufs=1))
            r1neg = pneg.tile([128, 6, OWN], bf16, name="r1neg")
            for kb in range(6):
                nc.vector.tensor_scalar(r1neg[:, kb, :], r1r[:, 6 + kb, :],
                                        -1.0, None, op0=ALU.mult)
            ph = es_f.enter_context(tc.tile_pool(name="ph", bufs=1))
            phn = es_f.enter_context(tc.tile_pool(name="phn", bufs=1))
            pw1 = es_f.enter_context(tc.tile_pool(name="pw1", bufs=3))
            pw2 = es_f.enter_context(tc.tile_pool(name="pw2", bufs=4))
            pscf = es_f.enter_context(tc.tile_pool(name="pscf", bufs=4))
            pout = es_f.enter_context(tc.tile_pool(name="pout", bufs=2))
            ps_f = es_f.enter_context(
                tc.tile_pool(name="ps_f", bufs=6, space="PSUM"))
            for cc in range(2):
                cs = slice(cc * MC, cc * MC + MC)
                h_t = ph.tile([128, 48, MC], bf16, tag="h", name=f"h{cc}")
                hn_t = phn.tile([128, 24, MC], bf16, tag="hn", name=f"hn{cc}")
                for Cb in range(24):
                    w1r_t = pw1.tile([128, 6, 128], bf16, tag="w1r",
                                     name=f"w1r{cc}_{Cb}")
                    w1i_t = pw1.tile([128, 6, 128], bf16, tag="w1i",
                                     name=f"w1i{cc}_{Cb}")
                    w1s_t = pw1.tile([4, 256], bf16, tag="w1s",
                                     name=f"w1s{cc}_{Cb}")
                    nc.sync.dma_start(w1r_t[:], w_fc1r[Cb]
                                      .rearrange("b p n -> p b n"))
                    nc.sync.dma_start(w1i_t[:], w_fc1i[Cb]
                                      .rearrange("b p n -> p b n"))
                    nc.sync.dma_start(w1s_t[:], w_fc1_s[Cb])
                    hr_ps = ps_f.tile([128, MC], f32, tag="fps",
                                      name=f"hrps{cc}_{Cb}")
                    hi_ps = ps_f.tile([128, MC], f32, tag="fps",
                                      name=f"hips{cc}_{Cb}")
                    for kb in range(6):
                        st = (kb == 0)
                        nc.tensor.matmul(hr_ps[:], w1r_t[:, kb, :],
                                         r1r[:, kb, cs], start=st, stop=False)
                        nc.tensor.matmul(hi_ps[:], w1i_t[:, kb, :],
                                         r1r[:, kb, cs], start=st, stop=False)
                    for kb in range(6):
                        nc.tensor.matmul(hr_ps[:], w1i_t[:, kb, :],
                                         r1neg[:, kb, cs], start=False, stop=False)
                        nc.tensor.matmul(hi_ps[:], w1r_t[:, kb, :],
                                         r1r[:, 6 + kb, cs], start=False,
                                         stop=False)
                    nc.tensor.matmul(hr_ps[:], w1s_t[:, 0:128], stat2[:, cs],
                                     start=False, stop=True)
                    nc.tensor.matmul(hi_ps[:], w1s_t[:, 128:256], stat2[:, cs],
                                     start=False, stop=True)
                    gr = pscf.tile([128, MC], f32, tag="g", name=f"gr{cc}_{Cb}")
                    gi = pscf.tile([128, MC], f32, tag="g", name=f"gi{cc}_{Cb}")
                    nc.vector.tensor_tensor(gr[:], hr_ps[:], rstd2_bc[:, cs],
                                            op=ALU.mult)
                    nc.vector.tensor_tensor(gi[:], hi_ps[:], rstd2_bc[:, cs],
                                            op=ALU.mult)
                    nc.scalar.activation(h_t[:, Cb, :], gr[:], AF.Gelu)
                    nc.scalar.activation(h_t[:, 24 + Cb, :], gi[:], AF.Gelu)
                    nc.vector.tensor_scalar(hn_t[:, Cb, :], h_t[:, 24 + Cb, :],
                                            -1.0, None, op0=ALU.mult)
                for j in range(6):
                    w2r_a = pw2.tile([128, 12, 128], bf16, tag="w2",
                                     name=f"w2ra{cc}_{j}")
                    w2r_b = pw2.tile([128, 12, 128], bf16, tag="w2",
                                     name=f"w2rb{cc}_{j}")
                    w2i_a = pw2.tile([128, 12, 128], bf16, tag="w2",
                                     name=f"w2ia{cc}_{j}")
                    w2i_b = pw2.tile([128, 12, 128], bf16, tag="w2",
                                     name=f"w2ib{cc}_{j}")
                    w2s_t = pw2.tile([4, 256], bf16, tag="w2s",
                                     name=f"w2s{cc}_{j}")
                    nc.sync.dma_start(w2r_a[:], w_fc2r[j, 0:12]
                                      .rearrange("b p n -> p b n"))
                    nc.sync.dma_start(w2r_b[:], w_fc2r[j, 12:24]
                                      .rearrange("b p n -> p b n"))
                    nc.sync.dma_start(w2i_a[:], w_fc2i[j, 0:12]
                                      .rearrange("b p n -> p b n"))
                    nc.sync.dma_start(w2i_b[:], w_fc2i[j, 12:24]
                                      .rearrange("b p n -> p b n"))
                    nc.sync.dma_start(w2s_t[:], w_fc2_s[j])
                    or_ps = ps_f.tile([128, MC], f32, tag="fps",
                                      name=f"orps{cc}_{j}")
                    oi_ps = ps_f.tile([128, MC], f32, tag="fps",
                                      name=f"oips{cc}_{j}")
                    for kb in range(24):
                        w2r = w2r_a[:, kb, :] if kb < 12 else w2r_b[:, kb - 12, :]
                        w2i = w2i_a[:, kb, :] if kb < 12 else w2i_b[:, kb - 12, :]
                        st = (kb == 0)
                        nc.tensor.matmul(or_ps[:], w2r, h_t[:, kb, :],
                                         start=st, stop=False)
                        nc.tensor.matmul(oi_ps[:], w2i, h_t[:, kb, :],
                                         start=st, stop=False)
                    for kb in range(24):
                        w2r = w2r_a[:, kb, :] if kb < 12 else w2r_b[:, kb - 12, :]
                        w2i = w2i_a[:, kb, :] if kb < 12 else w2i_b[:, kb - 12, :]
                        nc.tensor.matmul(or_ps[:], w2i, hn_t[:, kb, :],
                                         start=False, stop=False)
                        nc.tensor.matmul(oi_ps[:], w2r,
                                         h_t[:, 24 + kb, :],
                                         start=False, stop=False)
                    nc.tensor.matmul(or_ps[:], w2s_t[:, 0:128],
                                     t_stat_one[:, cs], start=False, stop=True)
                    nc.tensor.matmul(oi_ps[:], w2s_t[:, 128:256],
                                     t_stat_one[:, cs], start=False, stop=True)
                    o_r = pout.tile([128, MC], bf16, tag="o", name=f"or{cc}_{j}")
                    o_i = pout.tile([128, MC], bf16, tag="o", name=f"oi{cc}_{j}")
                    nc.vector.tensor_tensor(o_r[:], or_ps[:], xr1[:, j, cs],
                                            op=ALU.add)
                    nc.vector.tensor_tensor(o_i[:], oi_ps[:], xr1[:, 6 + j, cs],
                                            op=ALU.add)
                    nc.sync.dma_start(out_fm[j, :, cs], o_r[:])
                    nc.sync.dma_start(out_fm[6 + j, :, cs], o_i[:])
    nc.compile()
    return nc


# --------------------------------------------------------------------------
# host side
# --------------------------------------------------------------------------

def _cx(a):
    return a[..., 0].astype(np.float64) + 1j * a[..., 1].astype(np.float64)


def _kcols(Wp, wsum, wb, plane, scale=1.0):
    """K-profile [1539, m] for output features with complex weight rows Wp
    [m, 768], LN fold sums wsum [m], bias-column wb [m]. K rows: xr(768),
    xi(768), mu_r, mu_i, std."""
    m = Wp.shape[0]
    out = np.zeros((1539, m), np.float64)
    if plane == "r":
        out[0:768] = Wp.real.T
        out[768:1536] = -Wp.imag.T
        out[1536] = -wsum.real
        out[1537] = wsum.imag
        out[1538] = wb.real
    else:
        out[0:768] = Wp.imag.T
        out[768:1536] = Wp.real.T
        out[1536] = -wsum.imag
        out[1537] = -wsum.real
        out[1538] = wb.imag
    return out * scale


def _prep_weights(inputs):
    n1 = _cx(inputs["n1_w"]); b1 = _cx(inputs["n1_b"])
    n2 = _cx(inputs["n2_w"]); b2 = _cx(inputs["n2_b"])
    Wqkv = _cx(inputs["qkv_w"])          # [2304, 768]
    Wp = _cx(inputs["proj_w"])           # [768, 768]
    bp = _cx(inputs["proj_b"])           # [768]
    W1 = _cx(inputs["fc1_w"])            # [3072, 768]
    bf1 = _cx(inputs["fc1_b"])           # [3072]
    W2 = _cx(inputs["fc2_w"])            # [768, 3072]
    bf2 = _cx(inputs["fc2_b"])           # [768]

    d = {}
    # ---- qkv (LN1-folded) ----
    Wq, Wk, Wv = Wqkv[0:768], Wqkv[768:1536], Wqkv[1536:2304]

    def fold1(W):
        Wf = W * n1[None, :]
        return Wf, Wf.sum(1), W @ b1

    w_qkv = np.zeros((H, 12, 128, 384), np.float32)
    w_qkv_s = np.zeros((H, 4, 384), np.float32)
    for h in range(H):
        rows = slice(h * DH, (h + 1) * DH)
        Qf, Qs, Qb = fold1(Wq[rows])
        Kf, Ks, Kb_ = fold1(Wk[rows])
        q1 = np.hstack([_kcols(Qf, Qs, Qb, "r", SCALE),
                        _kcols(Qf, Qs, Qb, "i", -SCALE)])
        q3 = np.hstack([_kcols(Qf, Qs, Qb, "i", SCALE),
                        _kcols(Qf, Qs, Qb, "r", SCALE)])
        kk = np.hstack([_kcols(Kf, Ks, Kb_, "r"), _kcols(Kf, Ks, Kb_, "i")])
        blk = np.hstack([q1, q3, kk]).astype(np.float32)       # [1539, 384]
        w_qkv[h] = blk[0:1536].reshape(12, 128, 384)
        w_qkv_s[h, 0:3] = blk[1536:1539]
    d["w_qkv"] = w_qkv
    d["w_qkv_s"] = w_qkv_s

    # ---- v (LN1-folded), rhs layout [K, 1536]; cols: pair*256+slot*128+plane*64+dh
    wv_full = np.zeros((1539, 1536), np.float64)
    for h in range(H):
        rows = slice(h * DH, (h + 1) * DH)
        Vf, Vs, Vb = fold1(Wv[rows])
        base = h * 128
        wv_full[:, base:base + 64] = _kcols(Vf, Vs, Vb, "r")
        wv_full[:, base + 64:base + 128] = _kcols(Vf, Vs, Vb, "i")
    d["w_v"] = wv_full[0:1536].reshape(12, 128, 1536).astype(np.float32)
    wvs = np.zeros((4, 1536), np.float32)
    wvs[0:3] = wv_full[1536:1539]
    d["w_v_s"] = wvs

    # ---- proj (plain + bias); K rows = attn features: per head [a_r(64); a_i(64)]
    w_proj = np.zeros((12, 12, 128, 128), np.float32)
    w_proj_s = np.zeros((12, 4, 128), np.float32)
    for opb in range(12):
        plane = "r" if opb < 6 else "i"
        orow = slice((opb % 6) * 128, (opb % 6) * 128 + 128)
        Wpo = Wp[orow]                               # [128, 768] complex
        prof = np.zeros((1536, 128), np.float64)
        for hh in range(H):
            cols = slice(hh * DH, (hh + 1) * DH)
            if plane == "r":
                prof[hh * 128:hh * 128 + 64] = Wpo.real[:, cols].T
                prof[hh * 128 + 64:hh * 128 + 128] = -Wpo.imag[:, cols].T
            else:
                prof[hh * 128:hh * 128 + 64] = Wpo.imag[:, cols].T
                prof[hh * 128 + 64:hh * 128 + 128] = Wpo.real[:, cols].T
        w_proj[opb] = prof.reshape(12, 128, 128)
        w_proj_s[opb, 0] = (bp.real if plane == "r" else bp.imag)[orow]
    d["w_proj"] = w_proj
    d["w_proj_s"] = w_proj_s

    # ---- fc1 (LN2-folded, shared-tile form) ----
    W1f = W1 * n2[None, :]
    W1s = W1f.sum(1)
    W1b = W1 @ b2 + bf1
    w_fc1r = np.zeros((24, 6, 128, 128), np.float32)
    w_fc1i = np.zeros((24, 6, 128, 128), np.float32)
    w_fc1_s = np.zeros((24, 4, 256), np.float32)
    for Cb in range(24):
        orow = slice(Cb * 128, (Cb + 1) * 128)
        for kb in range(6):
            icol = slice(kb * 128, (kb + 1) * 128)
            w_fc1r[Cb, kb] = W1f.real[orow, icol].T
            w_fc1i[Cb, kb] = W1f.imag[orow, icol].T
        w_fc1_s[Cb, 0, 0:128] = -W1s.real[orow]
        w_fc1_s[Cb, 1, 0:128] = W1s.imag[orow]
        w_fc1_s[Cb, 2, 0:128] = W1b.real[orow]
        w_fc1_s[Cb, 0, 128:256] = -W1s.imag[orow]
        w_fc1_s[Cb, 1, 128:256] = -W1s.real[orow]
        w_fc1_s[Cb, 2, 128:256] = W1b.imag[orow]
    d["w_fc1r"] = w_fc1r
    d["w_fc1i"] = w_fc1i
    d["w_fc1_s"] = w_fc1_s

    # ---- fc2 (plain + bias) ----
    w_fc2r = np.zeros((6, 24, 128, 128), np.float32)
    w_fc2i = np.zeros((6, 24, 128, 128), np.float32)
    w_fc2_s = np.zeros((6, 4, 256), np.float32)
    for j in range(6):
        orow = slice(j * 128, (j + 1) * 128)
        for kb in range(24):
            icol = slice(kb * 128, (kb + 1) * 128)
            w_fc2r[j, kb] = W2.real[orow, icol].T
            w_fc2i[j, kb] = W2.imag[orow, icol].T
        w_fc2_s[j, 0, 0:128] = bf2.real[orow]
        w_fc2_s[j, 0, 128:256] = bf2.imag[orow]
    d["w_fc2r"] = w_fc2r
    d["w_fc2i"] = w_fc2i
    d["w_fc2_s"] = w_fc2_s

    # bf16 casts: gathered weights become per-core flat shards later
    for k in list(d.keys()):
        d[k] = d[k].astype(BF16)

    # ---- consts ----
    d["ones_col"] = np.ones((128, 1), BF16)
    oab = np.zeros((128, 4), np.float32)
    oab[:, 0] = 1.0 / C
    oab[:, 3] = 1.0 / C
    d["ones_ab"] = oab.astype(BF16)
    d["ones_s"] = np.full((128, 1), 1.0 / C, np.float32).astype(BF16)
    so = np.zeros((4, OWN), np.float32)
    so[0] = 1.0
    d["stat_one"] = so.astype(BF16)
    return d


_NC_CACHE = {}


def kernel(**inputs):
    if "nc" not in _NC_CACHE:
        _NC_CACHE["nc"] = build_nc()
    nc = _NC_CACHE["nc"]

    wd = _prep_weights(inputs)
    x = np.asarray(inputs["x"], np.float32)          # [B, N, C, 2]

    # split gathered weights into per-core flat shards
    shard_arrs = {}
    for nm in W_SHAPES:
        flat = np.ascontiguousarray(wd.pop(nm)).reshape(8, 128, _shard_cols(nm))
        shard_arrs[f"sh_{nm}"] = flat

    in_maps = []
    for c in range(NCORES):
        b, half = divmod(c, 2)
        xr_ = x[b, :, :, 0].T                        # [768, 1024]
        xi_ = x[b, :, :, 1].T
        stack = np.concatenate([xr_, xi_], 0)        # [1536, 1024]
        if half == 1:
            stack = np.concatenate([stack[:, OWN:], stack[:, :OWN]], 1)
        m = dict(wd)
        for nm in W_SHAPES:
            m[f"sh_{nm}"] = shard_arrs[f"sh_{nm}"][c]
        m["x_r"] = stack.astype(BF16).reshape(12, 128, N)
        in_maps.append(m)

    res = run_bass_kernel_spmd(nc, in_maps, list(range(NCORES)))
    out = np.empty((B, N, C, 2), np.float32)
    for c in range(NCORES):
        b, half = divmod(c, 2)
        o = np.asarray(res.results[c]["out_fm"], dtype=np.float32)
        sl = slice(half * OWN, half * OWN + OWN)
        out[b, sl, :, 0] = o[0:6].reshape(768, OWN).T
        out[b, sl, :, 1] = o[6:12].reshape(768, OWN).T
    return out


# revision 27
# speedup vs baseline: 1.1113x; 1.1113x over previous
"""Complex transformer block (LN->attn->LN->MLP, complex arithmetic) on 8 TRN2 cores.

Sharding: core c handles (batch b = c//2, sequence half = c%2). Weights are
shipped ONCE (sharded 1/8 per core, bf16, raw complex layout) and
redistributed on-device with AllGathers; the realified matmul "profile"
layouts (which duplicate/negate weight planes) are expanded on-device by DVE,
so no inflated weight bytes ever cross the host link. x ships once per core
(own 512 tokens) and the full batch sequence is reassembled on-device with a
pair AllGather; K/V use the canonical pair order (attention is permutation
invariant over key positions). Outputs return as bf16.

Layout: activations are feature-major [feature partition-blocks, tokens].
Complex tensors are realified as separate real/imag feature planes. LayerNorm
is fused into the following matmul: per-token stats (mu_r, mu_i, std) are
appended as 3 extra contraction rows with matching weight columns, and the
per-token rstd is applied by the PSUM-eviction multiply. Attention scores are
computed transposed ([t2, t1]) so softmax sums reduce via ones-matmuls, and V
is produced pre-transposed by swapping matmul operands. All matmuls run in
bf16 at full PE rate.
"""
import sys
sys.path.insert(0, "/opt/trn_rl_repo")

from contextlib import ExitStack

import numpy as np
import ml_dtypes

import concourse.bacc as bacc
import concourse.bass as bass
import concourse.mybir as mybir
import concourse.tile as tile
from concourse.bass_utils import run_bass_kernel_spmd

# Prefer the table set that covers the whole softmax chain (square+ln+exp)
# so the greedy act-table-load pass doesn't thrash sets on every block.
_orig_get_tables = bacc.get_activation_tables


def _reordered_tables(arch):
    t = _orig_get_tables(arch)
    keep = {"natural_log_exp_and_others", "gelu_and_others"}
    return {k: (v if k in keep else set()) for k, v in t.items()}


bacc.get_activation_tables = _reordered_tables

dt = mybir.dt
AF = mybir.ActivationFunctionType
ALU = mybir.AluOpType
BF16 = ml_dtypes.bfloat16

B, N, C, H, DH, HID = 4, 1024, 768, 12, 64, 3072
NCORES = 8
OWN = 512          # tokens per core
SCALE = DH ** -0.5
EPS = 1e-5
MC = 512           # MLP token chunk

# gathered (full) weight shapes, all bf16; shards are flat 1/8 slices.
# w_att planes: 0 qr, 1 qi, 2 kr, 3 ki, 4 vr, 5 vi (LN1-folded, transposed)
# w_proj planes: 0 r, 1 i (transposed blocks [j][h][64, 128])
W_SHAPES = {
    "w_att": (6, H, 6, 128, 64),
    "w_proj": (2, 6, 12, 64, 128),
    "w_fc1r": (24, 6, 128, 128),
    "w_fc1i": (24, 6, 128, 128),
    "w_fc2r": (6, 24, 128, 128),
    "w_fc2i": (6, 24, 128, 128),
}


def _shard_cols(name):
    n = int(np.prod(W_SHAPES[name]))
    assert n % (8 * 128) == 0
    return n // (8 * 128)


# --------------------------------------------------------------------------
# device program
# --------------------------------------------------------------------------

def build_nc(debug=False):
    nc = bacc.Bacc(trn_type="TRN2", target_bir_lowering=False, num_devices=8)
    f32 = dt.float32
    bf16 = dt.bfloat16

    # ---- DRAM I/O ----
    x_own = nc.dram_tensor("x_own", [12, 128, OWN], bf16, kind="ExternalInput")
    shards = {}
    for nm in W_SHAPES:
        shards[nm] = nc.dram_tensor(f"sh_{nm}", [128, _shard_cols(nm)], bf16,
                                    kind="ExternalInput")
    w_qkv_s = nc.dram_tensor("w_qkv_s", [H, 4, 384], bf16, kind="ExternalInput")
    w_v_s = nc.dram_tensor("w_v_s", [4, 1536], bf16, kind="ExternalInput")
    w_proj_s = nc.dram_tensor("w_proj_s", [12, 4, 128], bf16, kind="ExternalInput")
    w_fc1_s = nc.dram_tensor("w_fc1_s", [24, 4, 256], bf16, kind="ExternalInput")
    w_fc2_s = nc.dram_tensor("w_fc2_s", [6, 4, 256], bf16, kind="ExternalInput")
    ones_col = nc.dram_tensor("ones_col", [128, 1], bf16, kind="ExternalInput")
    ones_ab = nc.dram_tensor("ones_ab", [128, 4], bf16, kind="ExternalInput")
    ones_s = nc.dram_tensor("ones_s", [128, 1], bf16, kind="ExternalInput")
    stat_one = nc.dram_tensor("stat_one", [4, OWN], bf16, kind="ExternalInput")

    out_fm = nc.dram_tensor("out_fm", [12, 128, OWN], bf16, kind="ExternalOutput")
    dbg = {}
    if debug:
        for nm, shp, dtt in [
            ("d_xr", [128, 12, N], bf16), ("d_xo", [128, 12, OWN], bf16),
            ("d_stat1", [4, N], bf16), ("d_stato", [4, OWN], bf16),
            ("d_wq", [128, 12, 256], bf16), ("d_wk", [128, 12, 128], bf16),
            ("d_q", [128, 2, OWN], bf16), ("d_k", [128, N], bf16),
            ("d_vt", [128, 8, 256], bf16), ("d_et", [128, OWN], bf16),
            ("d_attn", [128, 12, OWN], bf16), ("d_rstdo", [128, 4], dt.float32),
        ]:
            dbg[nm] = nc.dram_tensor(nm, shp, dtt, kind="ExternalOutput")

    # gathered tensors (Shared scratchpad, written by AllGather)
    xg = nc.dram_tensor("xg", [2, 12, 128, OWN], bf16, kind="Internal")
    gath = {}
    for nm, shp in W_SHAPES.items():
        gath[nm] = nc.dram_tensor(f"g_{nm}", list(shp), bf16,
                                  kind="Internal", addr_space="Shared")
    w_att = gath["w_att"]
    w_proj = gath["w_proj"]
    w_fc1r = gath["w_fc1r"]
    w_fc1i = gath["w_fc1i"]
    w_fc2r = gath["w_fc2r"]
    w_fc2i = gath["w_fc2i"]

    with tile.TileContext(nc) as tc, ExitStack() as top:
        # ---- redistribution: bounce + AllGather, in order of use ----
        # (Tile tracks collective->consumer deps and emits staged waits on
        # the Collectives proc semaphore; verified in the compiled program.)
        gpool = top.enter_context(tc.tile_pool(name="gpool", bufs=1, space="DRAM"))
        xb = gpool.tile([12, 128, OWN], bf16, name="bn_x")
        nc.sync.dma_start(xb[:], x_own[:])
        nc.gpsimd.collective_compute(
            "AllGather", ALU.bypass,
            replica_groups=[[0, 1], [2, 3], [4, 5], [6, 7]],
            ins=[xb[:].opt()], outs=[xg[:].opt()])
        for nm in ["w_att", "w_proj", "w_fc1r", "w_fc1i", "w_fc2r", "w_fc2i"]:
            bounce = gpool.tile([128, _shard_cols(nm)], bf16, name=f"bn_{nm}")
            nc.sync.dma_start(bounce[:], shards[nm][:])
            nc.gpsimd.collective_compute(
                "AllGather", ALU.bypass,
                replica_groups=[list(range(8))],
                ins=[bounce[:].opt()], outs=[gath[nm][:].opt()])

        def gdma(nm, dst, src):
            return nc.sync.dma_start(dst, src)

        consts = top.enter_context(tc.tile_pool(name="consts", bufs=1))
        t_ones_col = consts.tile([128, 1], bf16)
        t_ones_ab = consts.tile([128, 4], bf16)
        t_ones_s = consts.tile([128, 1], bf16)
        t_stat_one = consts.tile([4, OWN], bf16)
        t_eps = consts.tile([1, 1], f32)
        nc.sync.dma_start(t_ones_col[:], ones_col[:])
        nc.sync.dma_start(t_ones_ab[:], ones_ab[:])
        nc.sync.dma_start(t_ones_s[:], ones_s[:])
        nc.sync.dma_start(t_stat_one[:], stat_one[:])
        nc.vector.memset(t_eps[:], EPS)

        poolR1 = top.enter_context(tc.tile_pool(name="poolR1", bufs=1))
        xr1 = poolR1.tile([128, 12, OWN], f32, name="xr1")

        with ExitStack() as es_x:
            poolX = es_x.enter_context(tc.tile_pool(name="poolX", bufs=1))
            xr = poolX.tile([128, 12, N], bf16, name="xr")
            xo = poolX.tile([128, 12, OWN], bf16, name="xo")
            pdram = es_x.enter_context(
                tc.tile_pool(name="pdram", bufs=1, space="DRAM"))
            rstd_dram = pdram.tile([1, N], f32, name="rstd_dram")
            stat1 = poolX.tile([4, N], bf16, name="stat1")
            rstd_bc1 = poolX.tile([128, N], f32, name="rstd_bc1")
            rstdT = poolX.tile([128, 8], f32, name="rstdT")
            stat_o = poolX.tile([4, OWN], bf16, name="stat_o")
            rstd_bc_o = poolX.tile([128, OWN], f32, name="rstd_bc_o")
            nc.vector.memset(stat1[:], 0.0)
            nc.vector.memset(stat_o[:], 0.0)
            for kb in range(12):
                nc.sync.dma_start(xo[:, kb, :], x_own[kb])
            for half in range(2):
                hs = slice(half * 512, half * 512 + 512)
                for kb in range(12):
                    nc.sync.dma_start(xr[:, kb, hs], xg[half, kb])
            # residual copy (bf16 -> f32)
            for kb in range(12):
                nc.vector.tensor_copy(xr1[:, kb, :], xo[:, kb, :])
            if debug:
                nc.sync.dma_start(dbg["d_xr"][:], xr[:])
                nc.sync.dma_start(dbg["d_xo"][:], xo[:])

            # ---------------- phase A: LN1 stats ---------------------------
            # ch 0/1: full canonical sequence (for K/V); ch 2: own tokens (Q)
            with ExitStack() as es_a:
                pa = es_a.enter_context(tc.tile_pool(name="pa_sb", bufs=12))
                pa_ps = es_a.enter_context(
                    tc.tile_pool(name="pa_ps", bufs=2, space="PSUM"))
                pa_sc = es_a.enter_context(tc.tile_pool(name="pa_sc", bufs=2))
                pdram_o = es_a.enter_context(
                    tc.tile_pool(name="pdram_o", bufs=1, space="DRAM"))
                sqs = []
                for kb in range(12):
                    sq = pa.tile([128, N], bf16, tag="sq", name=f"sq{kb}")
                    nc.scalar.activation(sq[:], xr[:, kb, :], AF.Square)
                    sqs.append(sq)
                sqos = []
                for kb in range(12):
                    sqo = pa.tile([128, OWN], bf16, tag="sqo", name=f"sqo{kb}")
                    nc.scalar.activation(sqo[:], xo[:, kb, :], AF.Square)
                    sqos.append(sqo)
                for ch in range(3):
                    own = ch == 2
                    sl = slice(0, 512) if own else slice(ch * 512, ch * 512 + 512)
                    src = xo if own else xr
                    sqsrc = sqos if own else sqs
                    mu_ps = pa_ps.tile([2, 512], f32, tag="mu", name=f"mu{ch}")
                    s_ps = pa_ps.tile([1, 512], f32, tag="s", name=f"s{ch}")
                    for kb in range(12):
                        lhs = t_ones_ab[:, 0:2] if kb < 6 else t_ones_ab[:, 2:4]
                        nc.tensor.matmul(mu_ps[:], lhs, src[:, kb, sl],
                                         start=(kb == 0), stop=(kb == 11))
                        nc.tensor.matmul(s_ps[:], t_ones_s[:], sqsrc[kb][:, sl],
                                         start=(kb == 0), stop=(kb == 11))
                    # var = S - mu_r^2 - mu_i^2 ; std = exp(.5 ln(var+eps))
                    mu_sb = pa_sc.tile([2, 512], f32, tag="musb", name=f"musb{ch}")
                    mu_fl = pa_sc.tile([1, 2, 512], f32, tag="mufl", name=f"mufl{ch}")
                    var = pa_sc.tile([1, 512], f32, tag="var", name=f"var{ch}")
                    lnv = pa_sc.tile([1, 512], f32, tag="lnv", name=f"lnv{ch}")
                    nc.vector.tensor_copy(mu_sb[:], mu_ps[:])
                    nc.sync.dma_start(mu_fl[:, 0, :], mu_sb[0:1, :])
                    nc.sync.dma_start(mu_fl[:, 1, :], mu_sb[1:2, :])
                    sq_mu = pa_sc.tile([1, 2, 512], f32, tag="sqmu", name=f"sqmu{ch}")
                    nc.vector.tensor_tensor(sq_mu[:], mu_fl[:], mu_fl[:],
                                            op=ALU.mult)
                    nc.vector.tensor_tensor(var[:], s_ps[:], sq_mu[:, 0, :],
                                            op=ALU.subtract)
                    nc.vector.tensor_tensor(var[:], var[:], sq_mu[:, 1, :],
                                            op=ALU.subtract)
                    nc.scalar.activation(lnv[:], var[:], AF.Ln, bias=t_eps[:])
                    # stats rows: 0=mu_r 1=mu_i 2=std
                    stt = stat_o if own else stat1
                    nc.vector.tensor_copy(stt[0:2, sl], mu_sb[:])
                    std_row = pa_sc.tile([1, 512], bf16, tag="stdr", name=f"stdr{ch}")
                    nc.scalar.activation(std_row[:], lnv[:], AF.Exp, scale=0.5)
                    nc.sync.dma_start(stt[2:3, sl], std_row[:])
                    rstd_row = pa_sc.tile([1, 512], f32, tag="rst", name=f"rst{ch}")
                    nc.scalar.activation(rstd_row[:], lnv[:], AF.Exp, scale=-0.5)
                    if own:
                        rstd_dram_o = pdram_o.tile([1, OWN], f32, name="rstd_dram_o")
                        nc.sync.dma_start(rstd_dram_o[:], rstd_row[:])
                        bco = bass.AP(tensor=rstd_dram_o.tensor,
                                      offset=rstd_dram_o[:].offset,
                                      ap=[[0, 128]] + rstd_dram_o[:].ap[1:])
                        nc.sync.dma_start(rstd_bc_o[:], bco)
                    else:
                        nc.sync.dma_start(rstd_dram[:, sl], rstd_row[:])
                        bcast = bass.AP(tensor=rstd_dram.tensor,
                                        offset=rstd_dram[:, sl].offset,
                                        ap=[[0, 128]] + rstd_dram[:, sl].ap[1:])
                        nc.sync.dma_start(rstd_bc1[:, sl], bcast)
                # rstd transposed: rstdT[p, t2b] = rstd[t2b*128 + p]
                nc.sync.dma_start(
                    rstdT[:],
                    rstd_dram[:].rearrange("o (b p) -> (o p) b", p=128))
                if debug:
                    nc.sync.dma_start(dbg["d_stat1"][:], stat1[:])
                    nc.sync.dma_start(dbg["d_stato"][:], stat_o[:])
                    nc.sync.dma_start(dbg["d_rstdo"][:], rstd_bc_o[:, 0:4])

            # ---------------- phase BC: qkv + attention per head ----------
            es_attn = ExitStack()
            attnp = es_attn.enter_context(tc.tile_pool(name="attnp", bufs=1))
            attn = attnp.tile([128, 12, OWN], bf16, name="attn")
            es_b = ExitStack()
            pq = es_b.enter_context(tc.tile_pool(name="pq", bufs=2))
            pk = es_b.enter_context(tc.tile_pool(name="pk", bufs=2))
            pvt = es_b.enter_context(tc.tile_pool(name="pvt", bufs=2))
            pwv = es_b.enter_context(tc.tile_pool(name="pwv", bufs=1))
            pwq = es_b.enter_context(tc.tile_pool(name="pwq", bufs=2))
            praw = es_b.enter_context(tc.tile_pool(name="praw", bufs=2))
            pet = es_b.enter_context(tc.tile_pool(name="pet", bufs=6))
            psc = es_b.enter_context(tc.tile_pool(name="psc", bufs=6))
            prd = es_b.enter_context(tc.tile_pool(name="prd", bufs=2))
            ps_sc = es_b.enter_context(
                tc.tile_pool(name="ps_sc", bufs=2, space="PSUM"))
            ps_qkv = es_b.enter_context(
                tc.tile_pool(name="ps_qkv", bufs=3, space="PSUM"))
            ps_acc = es_b.enter_context(
                tc.tile_pool(name="ps_acc", bufs=2, space="PSUM"))
            pdram_rd = es_b.enter_context(
                tc.tile_pool(name="pdram_rd", bufs=2, space="DRAM"))
            vt_pair = None
            et_fifo = []
            acc_ps = {}
            LAG = 4

            def emit_avden(ent):
                h2, t2b2, et2, vt2 = ent
                slot2 = h2 % 2
                if t2b2 == 0:
                    acc_ps[h2] = (
                        ps_acc.tile([128, OWN], f32, tag="av", name=f"av{h2}",
                                    bufs=2),
                        ps_acc.tile([1, OWN], f32, tag="den", name=f"den{h2}",
                                    bufs=1),
                    )
                av2, den2 = acc_ps[h2]
                nc.tensor.matmul(den2[:], t_ones_col[:], et2[:],
                                 start=(t2b2 == 0), stop=(t2b2 == 7))
                dsl2 = slice(slot2 * 128, slot2 * 128 + 128)
                nc.tensor.matmul(av2[:], vt2[:, t2b2, dsl2], et2[:],
                                 start=(t2b2 == 0), stop=(t2b2 == 7))
                if t2b2 == 7:
                    den_sb = prd.tile([1, OWN], f32, tag="den_sb",
                                      name=f"dsb{h2}", bufs=1)
                    rd = prd.tile([1, OWN], f32, tag="rd", name=f"rd{h2}",
                                  bufs=1)
                    nc.vector.tensor_copy(den_sb[:], den2[:])
                    nc.vector.reciprocal(rd[:], den_sb[:])
                    rd_dram = pdram_rd.tile([1, OWN], f32, tag="rdd",
                                            name=f"rdd{h2}")
                    nc.sync.dma_start(rd_dram[:], rd[:])
                    rd_bc = prd.tile([128, OWN], f32, tag="rd_bc",
                                     name=f"rdbc{h2}", bufs=1)
                    rd_bcast_ap = bass.AP(tensor=rd_dram.tensor,
                                          offset=rd_dram[:].offset,
                                          ap=[[0, 128]] + rd_dram[:].ap[1:])
                    nc.sync.dma_start(rd_bc[:], rd_bcast_ap)
                    nc.vector.tensor_tensor(attn[:, h2, :], av2[:], rd_bc[:],
                                            op=ALU.mult)
                    del acc_ps[h2]

            for h in range(H):
                pair, slot = divmod(h, 2)
                # raw LN1-folded weights for head h
                rq_r = praw.tile([128, 6, 64], bf16, tag="rqr", name=f"rqr{h}")
                rq_i = praw.tile([128, 6, 64], bf16, tag="rqi", name=f"rqi{h}")
                rk_r = praw.tile([128, 6, 64], bf16, tag="rkr", name=f"rkr{h}")
                rk_i = praw.tile([128, 6, 64], bf16, tag="rki", name=f"rki{h}")
                nc.sync.dma_start(rq_r[:], w_att[0, h].rearrange("k p n -> p k n"))
                nc.sync.dma_start(rq_i[:], w_att[1, h].rearrange("k p n -> p k n"))
                nc.sync.dma_start(rk_r[:], w_att[2, h].rearrange("k p n -> p k n"))
                nc.sync.dma_start(rk_i[:], w_att[3, h].rearrange("k p n -> p k n"))
                # expand to profile layout:
                # wq_t[:, kb, 0:128] = q1 = [q_r; -q_i] cols, [:, kb, 128:256] = q3
                # wk_t[:, kb, :]     = k  = [k_r; k_i] cols
                wq_t = pwq.tile([128, 12, 256], bf16, tag="wq", name=f"wq{h}")
                wk_t = pwq.tile([128, 12, 128], bf16, tag="wk", name=f"wk{h}")

                def ts(dst, src, mul, _nc=nc):
                    _nc.vector.tensor_scalar(dst, src, mul, None, op0=ALU.mult)

                ts(wq_t[:, 0:6, 0:64], rq_r[:], SCALE)
                ts(wq_t[:, 6:12, 0:64], rq_i[:], -SCALE)
                ts(wq_t[:, 0:6, 64:128], rq_i[:], -SCALE)
                ts(wq_t[:, 6:12, 64:128], rq_r[:], -SCALE)
                ts(wq_t[:, 0:6, 128:192], rq_i[:], SCALE)
                ts(wq_t[:, 6:12, 128:192], rq_r[:], SCALE)
                ts(wq_t[:, 0:6, 192:256], rq_r[:], SCALE)
                ts(wq_t[:, 6:12, 192:256], rq_i[:], -SCALE)
                ts(wk_t[:, 0:6, 0:64], rk_r[:], 1.0)
                ts(wk_t[:, 6:12, 0:64], rk_i[:], -1.0)
                ts(wk_t[:, 0:6, 64:128], rk_i[:], 1.0)
                ts(wk_t[:, 6:12, 64:128], rk_r[:], 1.0)

                q_t = pq.tile([128, 2, OWN], bf16, tag="q", name=f"q{h}")
                k_t = pk.tile([128, N], bf16, tag="k", name=f"k{h}")
                wqs_t = pwq.tile([4, 384], bf16, tag="wqs", name=f"wqs{h}")
                nc.sync.dma_start(wqs_t[:], w_qkv_s[h])
                q1_ps = ps_qkv.tile([128, OWN], f32, tag="qkv", name=f"q1ps{h}")
                q3_ps = ps_qkv.tile([128, OWN], f32, tag="qkv", name=f"q3ps{h}")
                for kb in range(12):
                    st = (kb == 0)
                    nc.tensor.matmul(q1_ps[:], wq_t[:, kb, 0:128],
                                     xo[:, kb, :], start=st, stop=False)
                    nc.tensor.matmul(q3_ps[:], wq_t[:, kb, 128:256],
                                     xo[:, kb, :], start=st, stop=False)
                nc.tensor.matmul(q1_ps[:], wqs_t[:, 0:128], stat_o[:],
                                 start=False, stop=True)
                nc.tensor.matmul(q3_ps[:], wqs_t[:, 128:256], stat_o[:],
                                 start=False, stop=True)
                nc.vector.tensor_tensor(q_t[:, 0, :], q1_ps[:],
                                        rstd_bc_o[:], op=ALU.mult)
                nc.vector.tensor_tensor(q_t[:, 1, :], q3_ps[:],
                                        rstd_bc_o[:], op=ALU.mult)
                k0_ps = ps_qkv.tile([128, 512], f32, tag="qkv", name=f"k0ps{h}")
                k1_ps = ps_qkv.tile([128, 512], f32, tag="qkv", name=f"k1ps{h}")
                for kb in range(12):
                    st = (kb == 0)
                    nc.tensor.matmul(k0_ps[:], wk_t[:, kb, :],
                                     xr[:, kb, 0:512], start=st, stop=False)
                    nc.tensor.matmul(k1_ps[:], wk_t[:, kb, :],
                                     xr[:, kb, 512:N], start=st, stop=False)
                nc.tensor.matmul(k0_ps[:], wqs_t[:, 256:384], stat1[:, 0:512],
                                 start=False, stop=True)
                nc.tensor.matmul(k1_ps[:], wqs_t[:, 256:384], stat1[:, 512:N],
                                 start=False, stop=True)
                nc.vector.tensor_tensor(k_t[:, 0:512], k0_ps[:],
                                        rstd_bc1[:, 0:512], op=ALU.mult)
                nc.vector.tensor_tensor(k_t[:, 512:N], k1_ps[:],
                                        rstd_bc1[:, 512:N], op=ALU.mult)
                if debug and h == 0:
                    nc.sync.dma_start(dbg["d_wq"][:], wq_t[:])
                    nc.sync.dma_start(dbg["d_wk"][:], wk_t[:])
                    nc.sync.dma_start(dbg["d_q"][:], q_t[:])
                    nc.sync.dma_start(dbg["d_k"][:], k_t[:])
                if slot == 0:
                    # V^T for this head pair: [t2, d] via swapped operands
                    wv_t = pwv.tile([128, 12, 256], bf16, tag="wv",
                                    name=f"wv{pair}")
                    wv_s = pwv.tile([4, 256], bf16, tag="wvs",
                                    name=f"wvs{pair}")
                    csl = slice(pair * 256, pair * 256 + 256)
                    nc.sync.dma_start(wv_s[:], w_v_s[:, csl])
                    for s2 in range(2):
                        h3 = 2 * pair + s2
                        rv_r = praw.tile([128, 6, 64], bf16, tag="rvr",
                                         name=f"rvr{pair}_{s2}")
                        rv_i = praw.tile([128, 6, 64], bf16, tag="rvi",
                                         name=f"rvi{pair}_{s2}")
                        nc.sync.dma_start(rv_r[:],
                                          w_att[4, h3].rearrange("k p n -> p k n"))
                        nc.sync.dma_start(rv_i[:],
                                          w_att[5, h3].rearrange("k p n -> p k n"))
                        cs2 = s2 * 128
                        ts(wv_t[:, 0:6, cs2 + 0:cs2 + 64], rv_r[:], 1.0)
                        ts(wv_t[:, 6:12, cs2 + 0:cs2 + 64], rv_i[:], -1.0)
                        ts(wv_t[:, 0:6, cs2 + 64:cs2 + 128], rv_i[:], 1.0)
                        ts(wv_t[:, 6:12, cs2 + 64:cs2 + 128], rv_r[:], 1.0)
                    vt_pair = pvt.tile([128, 8, 256], bf16, tag="vt",
                                       name=f"vt{pair}")
                    for t2b in range(8):
                        t2s = slice(t2b * 128, t2b * 128 + 128)
                        vt_ps = ps_qkv.tile([128, 256], f32, tag="qkv",
                                            name=f"vtps{pair}_{t2b}")
                        for kb in range(12):
                            nc.tensor.matmul(vt_ps[:], xr[:, kb, t2s],
                                             wv_t[:, kb, :],
                                             start=(kb == 0), stop=False)
                        nc.tensor.matmul(vt_ps[:], stat1[:, t2s], wv_s[:],
                                         start=False, stop=True)
                        nc.vector.tensor_scalar(
                            vt_pair[:, t2b, :], vt_ps[:],
                            rstdT[:, t2b:t2b + 1], None, op0=ALU.mult)
                    if debug and pair == 0:
                        nc.sync.dma_start(dbg["d_vt"][:], vt_pair[:])
                # scores + exp chain; den/av matmuls lag by LAG blocks
                for t2b in range(8):
                    t2s = slice(t2b * 128, t2b * 128 + 128)
                    sr_ps = ps_sc.tile([128, OWN], f32, tag="sc",
                                       name=f"sr{h}_{t2b}")
                    si_ps = ps_sc.tile([128, OWN], f32, tag="sc",
                                       name=f"si{h}_{t2b}")
                    nc.tensor.matmul(sr_ps[:], k_t[:, t2s], q_t[:, 0, :],
                                     start=True, stop=True)
                    nc.tensor.matmul(si_ps[:], k_t[:, t2s], q_t[:, 1, :],
                                     start=True, stop=True)
                    sqr = psc.tile([128, OWN], f32, tag="sc",
                                   name=f"sqr{h}_{t2b}")
                    sqi = psc.tile([128, OWN], f32, tag="sc",
                                   name=f"sqi{h}_{t2b}")
                    nc.scalar.activation(sqr[:], sr_ps[:], AF.Square)
                    nc.scalar.activation(sqi[:], si_ps[:], AF.Square)
                    # in-place chain on sqr: m2 -> ln -> 0.5ln -> mag
                    nc.vector.tensor_tensor(sqr[:], sqr[:], sqi[:], op=ALU.add)
                    nc.scalar.activation(sqr[:], sqr[:], AF.Ln)
                    nc.scalar.activation(sqr[:], sqr[:], AF.Exp, scale=0.5)
                    et = pet.tile([128, OWN], bf16, tag="et",
                                  name=f"et{h}_{t2b}")
                    nc.scalar.activation(et[:], sqr[:], AF.Exp)
                    if debug and h == 0 and t2b == 0:
                        nc.sync.dma_start(dbg["d_et"][:], et[:])
                    et_fifo.append((h, t2b, et, vt_pair))
                    while len(et_fifo) > LAG:
                        emit_avden(et_fifo.pop(0))
            for ent in et_fifo:
                emit_avden(ent)
            et_fifo.clear()
            if debug:
                nc.sync.dma_start(dbg["d_attn"][:], attn[:])
            es_b.close()

            # ------------- phase D: proj + residual --------------------
            r1r = poolR1.tile([128, 12, OWN], bf16, name="r1r")
            with ExitStack() as es_d:
                pwp = es_d.enter_context(tc.tile_pool(name="pwp", bufs=3))
                ps_d = es_d.enter_context(
                    tc.tile_pool(name="ps_d", bufs=4, space="PSUM"))
                for opb in range(12):
                    rp_r = pwp.tile([64, 12, 128], bf16, tag="rpr",
                                    name=f"rpr{opb}")
                    rp_i = pwp.tile([64, 12, 128], bf16, tag="rpi",
                                    name=f"rpi{opb}")
                    j = opb % 6
                    nc.sync.dma_start(rp_r[:],
                                      w_proj[0, j].rearrange("k p n -> p k n"))
                    nc.sync.dma_start(rp_i[:],
                                      w_proj[1, j].rearrange("k p n -> p k n"))
                    wp_t = pwp.tile([128, 12, 128], bf16, tag="wp",
                                    name=f"wp{opb}")
                    if opb < 6:
                        nc.vector.tensor_scalar(wp_t[0:64, :, :], rp_r[:],
                                                1.0, None, op0=ALU.mult)
                        nc.vector.tensor_scalar(wp_t[64:128, :, :], rp_i[:],
                                                -1.0, None, op0=ALU.mult)
                    else:
                        nc.vector.tensor_scalar(wp_t[0:64, :, :], rp_i[:],
                                                1.0, None, op0=ALU.mult)
                        nc.vector.tensor_scalar(wp_t[64:128, :, :], rp_r[:],
                                                1.0, None, op0=ALU.mult)
                    wps_t = pwp.tile([4, 128], bf16, tag="wps",
                                     name=f"wps{opb}")
                    nc.sync.dma_start(wps_t[:], w_proj_s[opb])
                    pr_ps = ps_d.tile([128, OWN], f32, tag="pr",
                                      name=f"prps{opb}")
                    for kb in range(12):
                        nc.tensor.matmul(pr_ps[:], wp_t[:, kb, :],
                                         attn[:, kb, :],
                                         start=(kb == 0), stop=False)
                    nc.tensor.matmul(pr_ps[:], wps_t[:], t_stat_one[:],
                                     start=False, stop=True)
                    nc.vector.tensor_tensor(xr1[:, opb, :], pr_ps[:],
                                            xr1[:, opb, :], op=ALU.add)
                    nc.vector.tensor_copy(r1r[:, opb, :], xr1[:, opb, :])
            es_attn.close()

        # ---------------- phase E: LN2 stats over own tokens --------------
        stat2 = poolR1.tile([4, OWN], bf16, name="stat2")
        rstd2_bc = poolR1.tile([128, OWN], f32, name="rstd2_bc")
        nc.vector.memset(stat2[:], 0.0)
        with ExitStack() as es_e:
            pe = es_e.enter_context(tc.tile_pool(name="pe_sb", bufs=1))
            pdram2 = es_e.enter_context(
                tc.tile_pool(name="pdram2", bufs=1, space="DRAM"))
            pe_ps = es_e.enter_context(
                tc.tile_pool(name="pe_ps", bufs=2, space="PSUM"))
            sq2s = []
            for kb in range(12):
                sq2 = pe.tile([128, OWN], bf16, tag="sq2", name=f"sq2_{kb}", bufs=12)
                nc.scalar.activation(sq2[:], r1r[:, kb, :], AF.Square)
                sq2s.append(sq2)
            mu2_ps = pe_ps.tile([2, OWN], f32, tag="mu2", name="mu2")
            s2_ps = pe_ps.tile([1, OWN], f32, tag="s2", name="s2")
            for kb in range(12):
                lhs = t_ones_ab[:, 0:2] if kb < 6 else t_ones_ab[:, 2:4]
                nc.tensor.matmul(mu2_ps[:], lhs, r1r[:, kb, :],
                                 start=(kb == 0), stop=(kb == 11))
                nc.tensor.matmul(s2_ps[:], t_ones_s[:], sq2s[kb][:],
                                 start=(kb == 0), stop=(kb == 11))
            mu2_sb = pe.tile([2, OWN], f32, tag="emusb", name="emusb")
            mu2_fl = pe.tile([1, 2, OWN], f32, tag="emufl", name="emufl")
            var = pe.tile([1, OWN], f32, tag="evar", name="evar")
            lnv = pe.tile([1, OWN], f32, tag="elnv", name="elnv")
            nc.vector.tensor_copy(mu2_sb[:], mu2_ps[:])
            nc.sync.dma_start(mu2_fl[:, 0, :], mu2_sb[0:1, :])
            nc.sync.dma_start(mu2_fl[:, 1, :], mu2_sb[1:2, :])
            sq_mu2 = pe.tile([1, 2, OWN], f32, tag="esqmu", name="esqmu")
            nc.vector.tensor_tensor(sq_mu2[:], mu2_fl[:], mu2_fl[:], op=ALU.mult)
            nc.vector.tensor_tensor(var[:], s2_ps[:], sq_mu2[:, 0, :],
                                    op=ALU.subtract)
            nc.vector.tensor_tensor(var[:], var[:], sq_mu2[:, 1, :],
                                    op=ALU.subtract)
            nc.scalar.activation(lnv[:], var[:], AF.Ln, bias=t_eps[:])
            nc.vector.tensor_copy(stat2[0:2, :], mu2_sb[:])
            std2_row = pe.tile([1, OWN], bf16, tag="estd", name="estd")
            nc.scalar.activation(std2_row[:], lnv[:], AF.Exp, scale=0.5)
            nc.sync.dma_start(stat2[2:3, :], std2_row[:])
            rstd2_row = pe.tile([1, OWN], f32, tag="ers", name="ers")
            nc.scalar.activation(rstd2_row[:], lnv[:], AF.Exp, scale=-0.5)
            rstd2_dram = pdram2.tile([1, OWN], f32, name="rstd2_dram")
            nc.sync.dma_start(rstd2_dram[:], rstd2_row[:])
            bcast2 = bass.AP(tensor=rstd2_dram.tensor, offset=rstd2_dram[:].offset,
                             ap=[[0, 128]] + rstd2_dram[:].ap[1:])
            nc.sync.dma_start(rstd2_bc[:], bcast2)

        # ---------------- phase F: MLP per 256-token chunk ----------------
        with ExitStack() as es_f:
            pneg = es_f.enter_context(tc.tile_pool(name="pneg", bufs=1))
            r1neg = pneg.tile([128, 6, OWN], bf16, name="r1neg")
            for kb in range(6):
                nc.vector.tensor_scalar(r1neg[:, kb, :], r1r[:, 6 + kb, :],
                                        -1.0, None, op0=ALU.mult)
            ph = es_f.enter_context(tc.tile_pool(name="ph", bufs=1))
            phn = es_f.enter_context(tc.tile_pool(name="phn", bufs=1))
            pw1 = es_f.enter_context(tc.tile_pool(name="pw1", bufs=3))
            pw2 = es_f.enter_context(tc.tile_pool(name="pw2", bufs=8))
            pscf = es_f.enter_context(tc.tile_pool(name="pscf", bufs=4))
            pout = es_f.enter_context(tc.tile_pool(name="pout", bufs=2))
            ps_f = es_f.enter_context(
                tc.tile_pool(name="ps_f", bufs=6, space="PSUM"))
            for cc in range(OWN // MC):
                cs = slice(cc * MC, cc * MC + MC)
                h_t = ph.tile([128, 48, MC], bf16, tag="h", name=f"h{cc}")
                hn_t = phn.tile([128, 24, MC], bf16, tag="hn", name=f"hn{cc}")
                for Cb in range(24):
                    w1r_t = pw1.tile([128, 6, 128], bf16, tag="w1r",
                                     name=f"w1r{cc}_{Cb}")
                    w1i_t = pw1.tile([128, 6, 128], bf16, tag="w1i",
                                     name=f"w1i{cc}_{Cb}")
                    w1s_t = pw1.tile([4, 256], bf16, tag="w1s",
                                     name=f"w1s{cc}_{Cb}")
                    nc.sync.dma_start(w1r_t[:], w_fc1r[Cb]
                                      .rearrange("b p n -> p b n"))
                    nc.sync.dma_start(w1i_t[:], w_fc1i[Cb]
                                      .rearrange("b p n -> p b n"))
                    nc.sync.dma_start(w1s_t[:], w_fc1_s[Cb])
                    hr_ps = ps_f.tile([128, MC], f32, tag="fps",
                                      name=f"hrps{cc}_{Cb}")
                    hi_ps = ps_f.tile([128, MC], f32, tag="fps",
                                      name=f"hips{cc}_{Cb}")
                    for kb in range(6):
                        st = (kb == 0)
                        nc.tensor.matmul(hr_ps[:], w1r_t[:, kb, :],
                                         r1r[:, kb, cs], start=st, stop=False)
                        nc.tensor.matmul(hi_ps[:], w1i_t[:, kb, :],
                                         r1r[:, kb, cs], start=st, stop=False)
                    for kb in range(6):
                        nc.tensor.matmul(hr_ps[:], w1i_t[:, kb, :],
                                         r1neg[:, kb, cs], start=False, stop=False)
                        nc.tensor.matmul(hi_ps[:], w1r_t[:, kb, :],
                                         r1r[:, 6 + kb, cs], start=False,
                                         stop=False)
                    nc.tensor.matmul(hr_ps[:], w1s_t[:, 0:128], stat2[:, cs],
                                     start=False, stop=True)
                    nc.tensor.matmul(hi_ps[:], w1s_t[:, 128:256], stat2[:, cs],
                                     start=False, stop=True)
                    gr = pscf.tile([128, MC], f32, tag="g", name=f"gr{cc}_{Cb}")
                    gi = pscf.tile([128, MC], f32, tag="g", name=f"gi{cc}_{Cb}")
                    nc.vector.tensor_tensor(gr[:], hr_ps[:], rstd2_bc[:, cs],
                                            op=ALU.mult)
                    nc.vector.tensor_tensor(gi[:], hi_ps[:], rstd2_bc[:, cs],
                                            op=ALU.mult)
                    nc.scalar.activation(h_t[:, Cb, :], gr[:], AF.Gelu)
                    nc.scalar.activation(h_t[:, 24 + Cb, :], gi[:], AF.Gelu)
                    nc.vector.tensor_scalar(hn_t[:, Cb, :], h_t[:, 24 + Cb, :],
                                            -1.0, None, op0=ALU.mult)
                for j in range(6):
                    w2r_a = pw2.tile([128, 12, 128], bf16, tag="w2",
                                     name=f"w2ra{cc}_{j}")
                    w2r_b = pw2.tile([128, 12, 128], bf16, tag="w2",
                                     name=f"w2rb{cc}_{j}")
                    w2i_a = pw2.tile([128, 12, 128], bf16, tag="w2",
                                     name=f"w2ia{cc}_{j}")
                    w2i_b = pw2.tile([128, 12, 128], bf16, tag="w2",
                                     name=f"w2ib{cc}_{j}")
                    w2s_t = pw2.tile([4, 256], bf16, tag="w2s",
                                     name=f"w2s{cc}_{j}")
                    nc.sync.dma_start(w2r_a[:], w_fc2r[j, 0:12]
                                      .rearrange("b p n -> p b n"))
                    nc.sync.dma_start(w2r_b[:], w_fc2r[j, 12:24]
                                      .rearrange("b p n -> p b n"))
                    nc.sync.dma_start(w2i_a[:], w_fc2i[j, 0:12]
                                      .rearrange("b p n -> p b n"))
                    nc.sync.dma_start(w2i_b[:], w_fc2i[j, 12:24]
                                      .rearrange("b p n -> p b n"))
                    nc.sync.dma_start(w2s_t[:], w_fc2_s[j])
                    or_ps = ps_f.tile([128, MC], f32, tag="fps",
                                      name=f"orps{cc}_{j}")
                    oi_ps = ps_f.tile([128, MC], f32, tag="fps",
                                      name=f"oips{cc}_{j}")
                    for kb in range(24):
                        w2r = w2r_a[:, kb, :] if kb < 12 else w2r_b[:, kb - 12, :]
                        w2i = w2i_a[:, kb, :] if kb < 12 else w2i_b[:, kb - 12, :]
                        st = (kb == 0)
                        nc.tensor.matmul(or_ps[:], w2r, h_t[:, kb, :],
                                         start=st, stop=False)
                        nc.tensor.matmul(oi_ps[:], w2i, h_t[:, kb, :],
                                         start=st, stop=False)
                    for kb in range(24):
                        w2r = w2r_a[:, kb, :] if kb < 12 else w2r_b[:, kb - 12, :]
                        w2i = w2i_a[:, kb, :] if kb < 12 else w2i_b[:, kb - 12, :]
                        nc.tensor.matmul(or_ps[:], w2i, hn_t[:, kb, :],
                                         start=False, stop=False)
                        nc.tensor.matmul(oi_ps[:], w2r,
                                         h_t[:, 24 + kb, :],
                                         start=False, stop=False)
                    nc.tensor.matmul(or_ps[:], w2s_t[:, 0:128],
                                     t_stat_one[:, cs], start=False, stop=True)
                    nc.tensor.matmul(oi_ps[:], w2s_t[:, 128:256],
                                     t_stat_one[:, cs], start=False, stop=True)
                    o_r = pout.tile([128, MC], bf16, tag="o", name=f"or{cc}_{j}")
                    o_i = pout.tile([128, MC], bf16, tag="o", name=f"oi{cc}_{j}")
                    nc.vector.tensor_tensor(o_r[:], or_ps[:], xr1[:, j, cs],
                                            op=ALU.add)
                    nc.vector.tensor_tensor(o_i[:], oi_ps[:], xr1[:, 6 + j, cs],
                                            op=ALU.add)
                    nc.sync.dma_start(out_fm[j, :, cs], o_r[:])
                    nc.sync.dma_start(out_fm[6 + j, :, cs], o_i[:])
    nc.compile()
    return nc


# --------------------------------------------------------------------------
# host side
# --------------------------------------------------------------------------

def _cx(a):
    return a[..., 0].astype(np.float64) + 1j * a[..., 1].astype(np.float64)


def _prep_weights(inputs):
    n1 = _cx(inputs["n1_w"]); b1 = _cx(inputs["n1_b"])
    n2 = _cx(inputs["n2_w"]); b2 = _cx(inputs["n2_b"])
    Wqkv = _cx(inputs["qkv_w"])          # [2304, 768]
    Wp = _cx(inputs["proj_w"])           # [768, 768]
    bp = _cx(inputs["proj_b"])           # [768]
    W1 = _cx(inputs["fc1_w"])            # [3072, 768]
    bf1 = _cx(inputs["fc1_b"])           # [3072]
    W2 = _cx(inputs["fc2_w"])            # [768, 3072]
    bf2 = _cx(inputs["fc2_b"])           # [768]

    d = {}
    # ---- qkv (LN1-folded), raw transposed blocks [H, 6, 128, 64] ----
    Wq, Wk, Wv = Wqkv[0:768], Wqkv[768:1536], Wqkv[1536:2304]

    def fold1(W):
        Wf = W * n1[None, :]
        return Wf, Wf.sum(1), W @ b1

    w_att = np.zeros((6, H, 6, 128, 64), np.float32)
    w_qkv_s = np.zeros((H, 4, 384), np.float32)
    wvs = np.zeros((4, 1536), np.float32)
    for h in range(H):
        rows = slice(h * DH, (h + 1) * DH)
        Qf, Qs, Qb = fold1(Wq[rows])
        Kf, Ks, Kb_ = fold1(Wk[rows])
        Vf, Vs, Vb = fold1(Wv[rows])
        for pi, M in enumerate([Qf.real, Qf.imag, Kf.real, Kf.imag,
                                Vf.real, Vf.imag]):
            w_att[pi, h] = M.T.reshape(6, 128, 64)
        # stat/bias columns (rows mu_r, mu_i, std of the K-profile)
        # q1 stats: kcols-r x S | kcols-i x -S ; q3: i x S | r x S ; k: r | i
        def stat3(ws, wb, plane, scale=1.0):
            out = np.zeros((3, DH), np.float64)
            if plane == "r":
                out[0] = -ws.real
                out[1] = ws.imag
                out[2] = wb.real
            else:
                out[0] = -ws.imag
                out[1] = -ws.real
                out[2] = wb.imag
            return out * scale

        w_qkv_s[h, 0:3, 0:64] = stat3(Qs, Qb, "r", SCALE)
        w_qkv_s[h, 0:3, 64:128] = stat3(Qs, Qb, "i", -SCALE)
        w_qkv_s[h, 0:3, 128:192] = stat3(Qs, Qb, "i", SCALE)
        w_qkv_s[h, 0:3, 192:256] = stat3(Qs, Qb, "r", SCALE)
        w_qkv_s[h, 0:3, 256:320] = stat3(Ks, Kb_, "r")
        w_qkv_s[h, 0:3, 320:384] = stat3(Ks, Kb_, "i")
        base = h * 128
        wvs[0:3, base:base + 64] = stat3(Vs, Vb, "r")
        wvs[0:3, base + 64:base + 128] = stat3(Vs, Vb, "i")
    d["w_att"] = w_att
    d["w_qkv_s"] = w_qkv_s
    d["w_v_s"] = wvs

    # ---- proj raw transposed blocks [2, 6, 12, 64, 128] + bias ----
    w_proj = np.zeros((2, 6, 12, 64, 128), np.float32)
    w_proj_s = np.zeros((12, 4, 128), np.float32)
    for j in range(6):
        orow = slice(j * 128, (j + 1) * 128)
        for hh in range(H):
            cols = slice(hh * DH, (hh + 1) * DH)
            w_proj[0, j, hh] = Wp.real[orow, cols].T
            w_proj[1, j, hh] = Wp.imag[orow, cols].T
    for opb in range(12):
        orow = slice((opb % 6) * 128, (opb % 6) * 128 + 128)
        w_proj_s[opb, 0] = (bp.real if opb < 6 else bp.imag)[orow]
    d["w_proj"] = w_proj
    d["w_proj_s"] = w_proj_s

    # ---- fc1 (LN2-folded, shared-tile form) ----
    W1f = W1 * n2[None, :]
    W1s = W1f.sum(1)
    W1b = W1 @ b2 + bf1
    w_fc1r = np.zeros((24, 6, 128, 128), np.float32)
    w_fc1i = np.zeros((24, 6, 128, 128), np.float32)
    w_fc1_s = np.zeros((24, 4, 256), np.float32)
    for Cb in range(24):
        orow = slice(Cb * 128, (Cb + 1) * 128)
        for kb in range(6):
            icol = slice(kb * 128, (kb + 1) * 128)
            w_fc1r[Cb, kb] = W1f.real[orow, icol].T
            w_fc1i[Cb, kb] = W1f.imag[orow, icol].T
        w_fc1_s[Cb, 0, 0:128] = -W1s.real[orow]
        w_fc1_s[Cb, 1, 0:128] = W1s.imag[orow]
        w_fc1_s[Cb, 2, 0:128] = W1b.real[orow]
        w_fc1_s[Cb, 0, 128:256] = -W1s.imag[orow]
        w_fc1_s[Cb, 1, 128:256] = -W1s.real[orow]
        w_fc1_s[Cb, 2, 128:256] = W1b.imag[orow]
    d["w_fc1r"] = w_fc1r
    d["w_fc1i"] = w_fc1i
    d["w_fc1_s"] = w_fc1_s

    # ---- fc2 (plain + bias) ----
    w_fc2r = np.zeros((6, 24, 128, 128), np.float32)
    w_fc2i = np.zeros((6, 24, 128, 128), np.float32)
    w_fc2_s = np.zeros((6, 4, 256), np.float32)
    for j in range(6):
        orow = slice(j * 128, (j + 1) * 128)
        for kb in range(24):
            icol = slice(kb * 128, (kb + 1) * 128)
            w_fc2r[j, kb] = W2.real[orow, icol].T
            w_fc2i[j, kb] = W2.imag[orow, icol].T
        w_fc2_s[j, 0, 0:128] = bf2.real[orow]
        w_fc2_s[j, 0, 128:256] = bf2.imag[orow]
    d["w_fc2r"] = w_fc2r
    d["w_fc2i"] = w_fc2i
    d["w_fc2_s"] = w_fc2_s

    for k in list(d.keys()):
        d[k] = d[k].astype(BF16)

    # ---- consts ----
    d["ones_col"] = np.ones((128, 1), BF16)
    oab = np.zeros((128, 4), np.float32)
    oab[:, 0] = 1.0 / C
    oab[:, 3] = 1.0 / C
    d["ones_ab"] = oab.astype(BF16)
    d["ones_s"] = np.full((128, 1), 1.0 / C, np.float32).astype(BF16)
    so = np.zeros((4, OWN), np.float32)
    so[0] = 1.0
    d["stat_one"] = so.astype(BF16)
    return d


def _make_in_maps(wd, x):
    shard_arrs = {}
    for nm in W_SHAPES:
        flat = np.ascontiguousarray(wd[nm]).reshape(8, 128, _shard_cols(nm))
        shard_arrs[f"sh_{nm}"] = flat
    smalls = {k: v for k, v in wd.items() if k not in W_SHAPES}
    in_maps = []
    for c in range(NCORES):
        b, half = divmod(c, 2)
        xr_ = x[b, :, :, 0].T                        # [768, 1024]
        xi_ = x[b, :, :, 1].T
        stack = np.concatenate([xr_, xi_], 0)        # [1536, 1024]
        own = stack[:, half * OWN:(half + 1) * OWN]
        m = dict(smalls)
        for nm in W_SHAPES:
            m[f"sh_{nm}"] = shard_arrs[f"sh_{nm}"][c]
        m["x_own"] = np.ascontiguousarray(own).astype(BF16).reshape(12, 128, OWN)
        in_maps.append(m)
    return in_maps


_NC_CACHE = {}


def kernel(**inputs):
    if "nc" not in _NC_CACHE:
        _NC_CACHE["nc"] = build_nc()
    nc = _NC_CACHE["nc"]

    wd = _prep_weights(inputs)
    x = np.asarray(inputs["x"], np.float32)          # [B, N, C, 2]
    in_maps = _make_in_maps(wd, x)

    res = run_bass_kernel_spmd(nc, in_maps, list(range(NCORES)))
    out = np.empty((B, N, C, 2), np.float32)
    for c in range(NCORES):
        b, half = divmod(c, 2)
        o = np.asarray(res.results[c]["out_fm"], dtype=np.float32)
        sl = slice(half * OWN, half * OWN + OWN)
        out[b, sl, :, 0] = o[0:6].reshape(768, OWN).T
        out[b, sl, :, 1] = o[6:12].reshape(768, OWN).T
    return out


# revision 28
# speedup vs baseline: 1.1952x; 1.0756x over previous
"""Complex transformer block (LN->attn->LN->MLP, complex arithmetic) on 8 TRN2 cores.

Sharding: core c handles (batch b = c//2, sequence half = c%2). Weights are
shipped ONCE (sharded 1/8 per core, bf16, raw complex layout) and
redistributed on-device with AllGathers; the realified matmul "profile"
layouts (which duplicate/negate weight planes) are expanded on-device by DVE,
so no inflated weight bytes ever cross the host link. x ships once per core
(own 512 tokens) and the full batch sequence is reassembled on-device with a
pair AllGather; K/V use the canonical pair order (attention is permutation
invariant over key positions). Outputs return as bf16.

Layout: activations are feature-major [feature partition-blocks, tokens].
Complex tensors are realified as separate real/imag feature planes. LayerNorm
is fused into the following matmul: per-token stats (mu_r, mu_i, std) are
appended as 3 extra contraction rows with matching weight columns, and the
per-token rstd is applied by the PSUM-eviction multiply. Attention scores are
computed transposed ([t2, t1]) so softmax sums reduce via ones-matmuls, and V
is produced pre-transposed by swapping matmul operands. All matmuls run in
bf16 at full PE rate.
"""
import sys
sys.path.insert(0, "/opt/trn_rl_repo")

from contextlib import ExitStack

import numpy as np
import ml_dtypes

import concourse.bacc as bacc
import concourse.bass as bass
import concourse.mybir as mybir
import concourse.tile as tile
from concourse.bass_utils import run_bass_kernel_spmd

# Prefer the table set that covers the whole softmax chain (square+ln+exp)
# so the greedy act-table-load pass doesn't thrash sets on every block.
_orig_get_tables = bacc.get_activation_tables


def _reordered_tables(arch):
    t = _orig_get_tables(arch)
    keep = {"natural_log_exp_and_others", "gelu_and_others"}
    return {k: (v if k in keep else set()) for k, v in t.items()}


bacc.get_activation_tables = _reordered_tables

dt = mybir.dt
AF = mybir.ActivationFunctionType
ALU = mybir.AluOpType
BF16 = ml_dtypes.bfloat16

B, N, C, H, DH, HID = 4, 1024, 768, 12, 64, 3072
NCORES = 8
OWN = 512          # tokens per core
SCALE = DH ** -0.5
EPS = 1e-5
MC = 512           # MLP token chunk

# gathered (full) weight shapes, all bf16; shards are flat 1/8 slices.
# w_att planes: 0 qr, 1 qi, 2 kr, 3 ki, 4 vr, 5 vi (LN1-folded, transposed)
# w_proj planes: 0 r, 1 i (transposed blocks [j][h][64, 128])
W_SHAPES = {
    "w_att": (6, H, 6, 128, 64),
    "w_proj": (2, 6, 12, 64, 128),
    "w_fc1r": (24, 6, 128, 128),
    "w_fc1i": (24, 6, 128, 128),
    "w_fc2r": (6, 24, 128, 128),
    "w_fc2i": (6, 24, 128, 128),
}


def _shard_cols(name):
    n = int(np.prod(W_SHAPES[name]))
    assert n % (8 * 128) == 0
    return n // (8 * 128)


# --------------------------------------------------------------------------
# device program
# --------------------------------------------------------------------------

def build_nc(debug=False):
    nc = bacc.Bacc(trn_type="TRN2", target_bir_lowering=False, num_devices=8)
    f32 = dt.float32
    bf16 = dt.bfloat16

    # ---- DRAM I/O ----
    x_own = nc.dram_tensor("x_own", [12, 128, OWN], bf16, kind="ExternalInput")
    shards = {}
    for nm in W_SHAPES:
        shards[nm] = nc.dram_tensor(f"sh_{nm}", [128, _shard_cols(nm)], bf16,
                                    kind="ExternalInput")
    w_qkv_s = nc.dram_tensor("w_qkv_s", [H, 4, 384], bf16, kind="ExternalInput")
    w_v_s = nc.dram_tensor("w_v_s", [4, 1536], bf16, kind="ExternalInput")
    w_proj_s = nc.dram_tensor("w_proj_s", [12, 4, 128], bf16, kind="ExternalInput")
    w_fc1_s = nc.dram_tensor("w_fc1_s", [24, 4, 256], bf16, kind="ExternalInput")
    w_fc2_s = nc.dram_tensor("w_fc2_s", [6, 4, 256], bf16, kind="ExternalInput")
    ones_col = nc.dram_tensor("ones_col", [128, 1], bf16, kind="ExternalInput")
    ones_ab = nc.dram_tensor("ones_ab", [128, 4], bf16, kind="ExternalInput")
    ones_s = nc.dram_tensor("ones_s", [128, 1], bf16, kind="ExternalInput")
    stat_one = nc.dram_tensor("stat_one", [4, OWN], bf16, kind="ExternalInput")

    out_fm = nc.dram_tensor("out_fm", [12, 128, OWN], bf16, kind="ExternalOutput")
    dbg = {}
    if debug:
        for nm, shp, dtt in [
            ("d_xr", [128, 12, N], bf16), ("d_xo", [128, 12, OWN], bf16),
            ("d_stat1", [4, N], bf16), ("d_stato", [4, OWN], bf16),
            ("d_wq", [128, 12, 256], bf16), ("d_wk", [128, 12, 128], bf16),
            ("d_q", [128, 2, OWN], bf16), ("d_k", [128, N], bf16),
            ("d_vt", [128, 8, 256], bf16), ("d_et", [128, OWN], bf16),
            ("d_attn", [128, 12, OWN], bf16), ("d_rstdo", [128, 4], dt.float32),
        ]:
            dbg[nm] = nc.dram_tensor(nm, shp, dtt, kind="ExternalOutput")

    # gathered tensors (Shared scratchpad, written by AllGather)
    xg = nc.dram_tensor("xg", [2, 12, 128, OWN], bf16, kind="Internal")
    gath = {}
    for nm, shp in W_SHAPES.items():
        gath[nm] = nc.dram_tensor(f"g_{nm}", list(shp), bf16,
                                  kind="Internal", addr_space="Shared")
    w_att = gath["w_att"]
    w_proj = gath["w_proj"]
    w_fc1r = gath["w_fc1r"]
    w_fc1i = gath["w_fc1i"]
    w_fc2r = gath["w_fc2r"]
    w_fc2i = gath["w_fc2i"]

    with tile.TileContext(nc) as tc, ExitStack() as top:
        # ---- redistribution: bounce + AllGather, in order of use ----
        # (Tile tracks collective->consumer deps and emits staged waits on
        # the Collectives proc semaphore; verified in the compiled program.)
        gpool = top.enter_context(tc.tile_pool(name="gpool", bufs=1, space="DRAM"))
        xb = gpool.tile([12, 128, OWN], bf16, name="bn_x")
        nc.sync.dma_start(xb[:], x_own[:])
        nc.gpsimd.collective_compute(
            "AllGather", ALU.bypass,
            replica_groups=[[0, 1], [2, 3], [4, 5], [6, 7]],
            ins=[xb[:].opt()], outs=[xg[:].opt()])
        for nm in ["w_att", "w_proj", "w_fc1r", "w_fc1i", "w_fc2r", "w_fc2i"]:
            bounce = gpool.tile([128, _shard_cols(nm)], bf16, name=f"bn_{nm}")
            nc.sync.dma_start(bounce[:], shards[nm][:])
            nc.gpsimd.collective_compute(
                "AllGather", ALU.bypass,
                replica_groups=[list(range(8))],
                ins=[bounce[:].opt()], outs=[gath[nm][:].opt()])

        def gdma(nm, dst, src):
            return nc.sync.dma_start(dst, src)

        consts = top.enter_context(tc.tile_pool(name="consts", bufs=1))
        t_ones_col = consts.tile([128, 1], bf16)
        t_ones_ab = consts.tile([128, 4], bf16)
        t_ones_s = consts.tile([128, 1], bf16)
        t_stat_one = consts.tile([4, OWN], bf16)
        t_eps = consts.tile([1, 1], f32)
        nc.sync.dma_start(t_ones_col[:], ones_col[:])
        nc.sync.dma_start(t_ones_ab[:], ones_ab[:])
        nc.sync.dma_start(t_ones_s[:], ones_s[:])
        nc.sync.dma_start(t_stat_one[:], stat_one[:])
        nc.vector.memset(t_eps[:], EPS)

        poolR1 = top.enter_context(tc.tile_pool(name="poolR1", bufs=1))
        xr1 = poolR1.tile([128, 12, OWN], f32, name="xr1")

        with ExitStack() as es_x:
            poolX = es_x.enter_context(tc.tile_pool(name="poolX", bufs=1))
            xr = poolX.tile([128, 12, N], bf16, name="xr")
            xo = poolX.tile([128, 12, OWN], bf16, name="xo")
            pdram = es_x.enter_context(
                tc.tile_pool(name="pdram", bufs=1, space="DRAM"))
            rstd_dram = pdram.tile([1, N], f32, name="rstd_dram")
            stat1 = poolX.tile([4, N], bf16, name="stat1")
            rstd_bc1 = poolX.tile([128, N], f32, name="rstd_bc1")
            rstdT = poolX.tile([128, 8], f32, name="rstdT")
            stat_o = poolX.tile([4, OWN], bf16, name="stat_o")
            rstd_bc_o = poolX.tile([128, OWN], f32, name="rstd_bc_o")
            nc.vector.memset(stat1[:], 0.0)
            nc.vector.memset(stat_o[:], 0.0)
            for kb in range(12):
                nc.sync.dma_start(xo[:, kb, :], x_own[kb])
            for half in range(2):
                hs = slice(half * 512, half * 512 + 512)
                for kb in range(12):
                    nc.sync.dma_start(xr[:, kb, hs], xg[half, kb])
            # residual copy (bf16 -> f32)
            for kb in range(12):
                nc.vector.tensor_copy(xr1[:, kb, :], xo[:, kb, :])
            if debug:
                nc.sync.dma_start(dbg["d_xr"][:], xr[:])
                nc.sync.dma_start(dbg["d_xo"][:], xo[:])

            # ---------------- phase A: LN1 stats ---------------------------
            # ch 0/1: full canonical sequence (for K/V); ch 2: own tokens (Q)
            with ExitStack() as es_a:
                pa = es_a.enter_context(tc.tile_pool(name="pa_sb", bufs=12))
                pa_ps = es_a.enter_context(
                    tc.tile_pool(name="pa_ps", bufs=2, space="PSUM"))
                pa_sc = es_a.enter_context(tc.tile_pool(name="pa_sc", bufs=2))
                pdram_o = es_a.enter_context(
                    tc.tile_pool(name="pdram_o", bufs=1, space="DRAM"))
                sqs = []
                for kb in range(12):
                    sq = pa.tile([128, N], bf16, tag="sq", name=f"sq{kb}")
                    nc.scalar.activation(sq[:], xr[:, kb, :], AF.Square)
                    sqs.append(sq)
                sqos = []
                for kb in range(12):
                    sqo = pa.tile([128, OWN], bf16, tag="sqo", name=f"sqo{kb}")
                    nc.scalar.activation(sqo[:], xo[:, kb, :], AF.Square)
                    sqos.append(sqo)
                for ch in range(3):
                    own = ch == 2
                    sl = slice(0, 512) if own else slice(ch * 512, ch * 512 + 512)
                    src = xo if own else xr
                    sqsrc = sqos if own else sqs
                    mu_ps = pa_ps.tile([2, 512], f32, tag="mu", name=f"mu{ch}")
                    s_ps = pa_ps.tile([1, 512], f32, tag="s", name=f"s{ch}")
                    for kb in range(12):
                        lhs = t_ones_ab[:, 0:2] if kb < 6 else t_ones_ab[:, 2:4]
                        nc.tensor.matmul(mu_ps[:], lhs, src[:, kb, sl],
                                         start=(kb == 0), stop=(kb == 11))
                        nc.tensor.matmul(s_ps[:], t_ones_s[:], sqsrc[kb][:, sl],
                                         start=(kb == 0), stop=(kb == 11))
                    # var = S - mu_r^2 - mu_i^2 ; std = exp(.5 ln(var+eps))
                    mu_sb = pa_sc.tile([2, 512], f32, tag="musb", name=f"musb{ch}")
                    mu_fl = pa_sc.tile([1, 2, 512], f32, tag="mufl", name=f"mufl{ch}")
                    var = pa_sc.tile([1, 512], f32, tag="var", name=f"var{ch}")
                    lnv = pa_sc.tile([1, 512], f32, tag="lnv", name=f"lnv{ch}")
                    nc.vector.tensor_copy(mu_sb[:], mu_ps[:])
                    nc.sync.dma_start(mu_fl[:, 0, :], mu_sb[0:1, :])
                    nc.sync.dma_start(mu_fl[:, 1, :], mu_sb[1:2, :])
                    sq_mu = pa_sc.tile([1, 2, 512], f32, tag="sqmu", name=f"sqmu{ch}")
                    nc.vector.tensor_tensor(sq_mu[:], mu_fl[:], mu_fl[:],
                                            op=ALU.mult)
                    nc.vector.tensor_tensor(var[:], s_ps[:], sq_mu[:, 0, :],
                                            op=ALU.subtract)
                    nc.vector.tensor_tensor(var[:], var[:], sq_mu[:, 1, :],
                                            op=ALU.subtract)
                    nc.scalar.activation(lnv[:], var[:], AF.Ln, bias=t_eps[:])
                    # stats rows: 0=mu_r 1=mu_i 2=std
                    stt = stat_o if own else stat1
                    nc.vector.tensor_copy(stt[0:2, sl], mu_sb[:])
                    std_row = pa_sc.tile([1, 512], bf16, tag="stdr", name=f"stdr{ch}")
                    nc.scalar.activation(std_row[:], lnv[:], AF.Exp, scale=0.5)
                    nc.sync.dma_start(stt[2:3, sl], std_row[:])
                    rstd_row = pa_sc.tile([1, 512], f32, tag="rst", name=f"rst{ch}")
                    nc.scalar.activation(rstd_row[:], lnv[:], AF.Exp, scale=-0.5)
                    if own:
                        rstd_dram_o = pdram_o.tile([1, OWN], f32, name="rstd_dram_o")
                        nc.sync.dma_start(rstd_dram_o[:], rstd_row[:])
                        bco = bass.AP(tensor=rstd_dram_o.tensor,
                                      offset=rstd_dram_o[:].offset,
                                      ap=[[0, 128]] + rstd_dram_o[:].ap[1:])
                        nc.sync.dma_start(rstd_bc_o[:], bco)
                    else:
                        nc.sync.dma_start(rstd_dram[:, sl], rstd_row[:])
                        bcast = bass.AP(tensor=rstd_dram.tensor,
                                        offset=rstd_dram[:, sl].offset,
                                        ap=[[0, 128]] + rstd_dram[:, sl].ap[1:])
                        nc.sync.dma_start(rstd_bc1[:, sl], bcast)
                # rstd transposed: rstdT[p, t2b] = rstd[t2b*128 + p]
                nc.sync.dma_start(
                    rstdT[:],
                    rstd_dram[:].rearrange("o (b p) -> (o p) b", p=128))
                if debug:
                    nc.sync.dma_start(dbg["d_stat1"][:], stat1[:])
                    nc.sync.dma_start(dbg["d_stato"][:], stat_o[:])
                    nc.sync.dma_start(dbg["d_rstdo"][:], rstd_bc_o[:, 0:4])

            # ---------------- phase BC: qkv + attention per head ----------
            es_attn = ExitStack()
            attnp = es_attn.enter_context(tc.tile_pool(name="attnp", bufs=1))
            attn = attnp.tile([128, 12, OWN], bf16, name="attn")
            es_b = ExitStack()
            pq = es_b.enter_context(tc.tile_pool(name="pq", bufs=2))
            pk = es_b.enter_context(tc.tile_pool(name="pk", bufs=2))
            pvt = es_b.enter_context(tc.tile_pool(name="pvt", bufs=2))
            pwv = es_b.enter_context(tc.tile_pool(name="pwv", bufs=1))
            pwq = es_b.enter_context(tc.tile_pool(name="pwq", bufs=2))
            praw = es_b.enter_context(tc.tile_pool(name="praw", bufs=2))
            pet = es_b.enter_context(tc.tile_pool(name="pet", bufs=8))
            psc = es_b.enter_context(tc.tile_pool(name="psc", bufs=6))
            prd = es_b.enter_context(tc.tile_pool(name="prd", bufs=2))
            ps_sc = es_b.enter_context(
                tc.tile_pool(name="ps_sc", bufs=2, space="PSUM"))
            ps_qkv = es_b.enter_context(
                tc.tile_pool(name="ps_qkv", bufs=4, space="PSUM"))
            ps_acc = es_b.enter_context(
                tc.tile_pool(name="ps_acc", bufs=2, space="PSUM"))
            pdram_rd = es_b.enter_context(
                tc.tile_pool(name="pdram_rd", bufs=2, space="DRAM"))
            vt_pair = None
            et_fifo = []
            acc_ps = {}
            LAG = 6

            def emit_avden(ent):
                h2, t2b2, et2, vt2 = ent
                slot2 = h2 % 2
                if t2b2 == 0:
                    acc_ps[h2] = (
                        ps_acc.tile([128, OWN], f32, tag="av", name=f"av{h2}",
                                    bufs=1),
                        ps_acc.tile([1, OWN], f32, tag="den", name=f"den{h2}",
                                    bufs=1),
                    )
                av2, den2 = acc_ps[h2]
                nc.tensor.matmul(den2[:], t_ones_col[:], et2[:],
                                 start=(t2b2 == 0), stop=(t2b2 == 7))
                dsl2 = slice(slot2 * 128, slot2 * 128 + 128)
                nc.tensor.matmul(av2[:], vt2[:, t2b2, dsl2], et2[:],
                                 start=(t2b2 == 0), stop=(t2b2 == 7))
                if t2b2 == 7:
                    den_sb = prd.tile([1, OWN], f32, tag="den_sb",
                                      name=f"dsb{h2}", bufs=1)
                    rd = prd.tile([1, OWN], f32, tag="rd", name=f"rd{h2}",
                                  bufs=1)
                    nc.vector.tensor_copy(den_sb[:], den2[:])
                    nc.vector.reciprocal(rd[:], den_sb[:])
                    rd_dram = pdram_rd.tile([1, OWN], f32, tag="rdd",
                                            name=f"rdd{h2}")
                    nc.sync.dma_start(rd_dram[:], rd[:])
                    rd_bc = prd.tile([128, OWN], f32, tag="rd_bc",
                                     name=f"rdbc{h2}", bufs=1)
                    rd_bcast_ap = bass.AP(tensor=rd_dram.tensor,
                                          offset=rd_dram[:].offset,
                                          ap=[[0, 128]] + rd_dram[:].ap[1:])
                    nc.sync.dma_start(rd_bc[:], rd_bcast_ap)
                    nc.vector.tensor_tensor(attn[:, h2, :], av2[:], rd_bc[:],
                                            op=ALU.mult)
                    del acc_ps[h2]

            for h in range(H):
                pair, slot = divmod(h, 2)
                # raw LN1-folded weights for head h
                rq_r = praw.tile([128, 6, 64], bf16, tag="rqr", name=f"rqr{h}")
                rq_i = praw.tile([128, 6, 64], bf16, tag="rqi", name=f"rqi{h}")
                rk_r = praw.tile([128, 6, 64], bf16, tag="rkr", name=f"rkr{h}")
                rk_i = praw.tile([128, 6, 64], bf16, tag="rki", name=f"rki{h}")
                nc.sync.dma_start(rq_r[:], w_att[0, h].rearrange("k p n -> p k n"))
                nc.sync.dma_start(rq_i[:], w_att[1, h].rearrange("k p n -> p k n"))
                nc.sync.dma_start(rk_r[:], w_att[2, h].rearrange("k p n -> p k n"))
                nc.sync.dma_start(rk_i[:], w_att[3, h].rearrange("k p n -> p k n"))
                # expand to profile layout:
                # wq_t[:, kb, 0:128] = q1 = [q_r; -q_i] cols, [:, kb, 128:256] = q3
                # wk_t[:, kb, :]     = k  = [k_r; k_i] cols
                wq_t = pwq.tile([128, 12, 256], bf16, tag="wq", name=f"wq{h}")
                wk_t = pwq.tile([128, 12, 128], bf16, tag="wk", name=f"wk{h}")

                def ts(dst, src, mul, _nc=nc):
                    _nc.vector.tensor_scalar(dst, src, mul, None, op0=ALU.mult)

                ts(wq_t[:, 0:6, 0:64], rq_r[:], SCALE)
                ts(wq_t[:, 6:12, 0:64], rq_i[:], -SCALE)
                ts(wq_t[:, 0:6, 64:128], rq_i[:], -SCALE)
                ts(wq_t[:, 6:12, 64:128], rq_r[:], -SCALE)
                ts(wq_t[:, 0:6, 128:192], rq_i[:], SCALE)
                ts(wq_t[:, 6:12, 128:192], rq_r[:], SCALE)
                ts(wq_t[:, 0:6, 192:256], rq_r[:], SCALE)
                ts(wq_t[:, 6:12, 192:256], rq_i[:], -SCALE)
                ts(wk_t[:, 0:6, 0:64], rk_r[:], 1.0)
                ts(wk_t[:, 6:12, 0:64], rk_i[:], -1.0)
                ts(wk_t[:, 0:6, 64:128], rk_i[:], 1.0)
                ts(wk_t[:, 6:12, 64:128], rk_r[:], 1.0)

                q_t = pq.tile([128, 2, OWN], bf16, tag="q", name=f"q{h}")
                k_t = pk.tile([128, N], bf16, tag="k", name=f"k{h}")
                wqs_t = pwq.tile([4, 384], bf16, tag="wqs", name=f"wqs{h}")
                nc.sync.dma_start(wqs_t[:], w_qkv_s[h])
                q1_ps = ps_qkv.tile([128, OWN], f32, tag="qkv", name=f"q1ps{h}")
                q3_ps = ps_qkv.tile([128, OWN], f32, tag="qkv", name=f"q3ps{h}")
                for kb in range(12):
                    st = (kb == 0)
                    nc.tensor.matmul(q1_ps[:], wq_t[:, kb, 0:128],
                                     xo[:, kb, :], start=st, stop=False)
                    nc.tensor.matmul(q3_ps[:], wq_t[:, kb, 128:256],
                                     xo[:, kb, :], start=st, stop=False)
                nc.tensor.matmul(q1_ps[:], wqs_t[:, 0:128], stat_o[:],
                                 start=False, stop=True)
                nc.tensor.matmul(q3_ps[:], wqs_t[:, 128:256], stat_o[:],
                                 start=False, stop=True)
                nc.vector.tensor_tensor(q_t[:, 0, :], q1_ps[:],
                                        rstd_bc_o[:], op=ALU.mult)
                nc.vector.tensor_tensor(q_t[:, 1, :], q3_ps[:],
                                        rstd_bc_o[:], op=ALU.mult)
                k0_ps = ps_qkv.tile([128, 512], f32, tag="qkv", name=f"k0ps{h}")
                k1_ps = ps_qkv.tile([128, 512], f32, tag="qkv", name=f"k1ps{h}")
                for kb in range(12):
                    st = (kb == 0)
                    nc.tensor.matmul(k0_ps[:], wk_t[:, kb, :],
                                     xr[:, kb, 0:512], start=st, stop=False)
                    nc.tensor.matmul(k1_ps[:], wk_t[:, kb, :],
                                     xr[:, kb, 512:N], start=st, stop=False)
                nc.tensor.matmul(k0_ps[:], wqs_t[:, 256:384], stat1[:, 0:512],
                                 start=False, stop=True)
                nc.tensor.matmul(k1_ps[:], wqs_t[:, 256:384], stat1[:, 512:N],
                                 start=False, stop=True)
                nc.vector.tensor_tensor(k_t[:, 0:512], k0_ps[:],
                                        rstd_bc1[:, 0:512], op=ALU.mult)
                nc.vector.tensor_tensor(k_t[:, 512:N], k1_ps[:],
                                        rstd_bc1[:, 512:N], op=ALU.mult)
                if debug and h == 0:
                    nc.sync.dma_start(dbg["d_wq"][:], wq_t[:])
                    nc.sync.dma_start(dbg["d_wk"][:], wk_t[:])
                    nc.sync.dma_start(dbg["d_q"][:], q_t[:])
                    nc.sync.dma_start(dbg["d_k"][:], k_t[:])
                if slot == 0:
                    # V^T for this head pair: [t2, d] via swapped operands
                    wv_t = pwv.tile([128, 12, 256], bf16, tag="wv",
                                    name=f"wv{pair}")
                    wv_s = pwv.tile([4, 256], bf16, tag="wvs",
                                    name=f"wvs{pair}")
                    csl = slice(pair * 256, pair * 256 + 256)
                    nc.sync.dma_start(wv_s[:], w_v_s[:, csl])
                    for s2 in range(2):
                        h3 = 2 * pair + s2
                        rv_r = praw.tile([128, 6, 64], bf16, tag="rvr",
                                         name=f"rvr{pair}_{s2}")
                        rv_i = praw.tile([128, 6, 64], bf16, tag="rvi",
                                         name=f"rvi{pair}_{s2}")
                        nc.sync.dma_start(rv_r[:],
                                          w_att[4, h3].rearrange("k p n -> p k n"))
                        nc.sync.dma_start(rv_i[:],
                                          w_att[5, h3].rearrange("k p n -> p k n"))
                        cs2 = s2 * 128
                        ts(wv_t[:, 0:6, cs2 + 0:cs2 + 64], rv_r[:], 1.0)
                        ts(wv_t[:, 6:12, cs2 + 0:cs2 + 64], rv_i[:], -1.0)
                        ts(wv_t[:, 0:6, cs2 + 64:cs2 + 128], rv_i[:], 1.0)
                        ts(wv_t[:, 6:12, cs2 + 64:cs2 + 128], rv_r[:], 1.0)
                    vt_pair = pvt.tile([128, 8, 256], bf16, tag="vt",
                                       name=f"vt{pair}")
                    for t2b in range(8):
                        t2s = slice(t2b * 128, t2b * 128 + 128)
                        vt_ps = ps_qkv.tile([128, 256], f32, tag="qkv",
                                            name=f"vtps{pair}_{t2b}")
                        for kb in range(12):
                            nc.tensor.matmul(vt_ps[:], xr[:, kb, t2s],
                                             wv_t[:, kb, :],
                                             start=(kb == 0), stop=False)
                        nc.tensor.matmul(vt_ps[:], stat1[:, t2s], wv_s[:],
                                         start=False, stop=True)
                        nc.vector.tensor_scalar(
                            vt_pair[:, t2b, :], vt_ps[:],
                            rstdT[:, t2b:t2b + 1], None, op0=ALU.mult)
                    if debug and pair == 0:
                        nc.sync.dma_start(dbg["d_vt"][:], vt_pair[:])
                # scores + exp chain; den/av matmuls lag by LAG blocks
                for t2b in range(8):
                    t2s = slice(t2b * 128, t2b * 128 + 128)
                    sr_ps = ps_sc.tile([128, OWN], f32, tag="sc",
                                       name=f"sr{h}_{t2b}")
                    si_ps = ps_sc.tile([128, OWN], f32, tag="sc",
                                       name=f"si{h}_{t2b}")
                    nc.tensor.matmul(sr_ps[:], k_t[:, t2s], q_t[:, 0, :],
                                     start=True, stop=True)
                    nc.tensor.matmul(si_ps[:], k_t[:, t2s], q_t[:, 1, :],
                                     start=True, stop=True)
                    sqr = psc.tile([128, OWN], f32, tag="sc",
                                   name=f"sqr{h}_{t2b}")
                    sqi = psc.tile([128, OWN], f32, tag="sc",
                                   name=f"sqi{h}_{t2b}")
                    nc.scalar.activation(sqr[:], sr_ps[:], AF.Square)
                    nc.scalar.activation(sqi[:], si_ps[:], AF.Square)
                    # in-place chain on sqr: m2 -> ln -> 0.5ln -> mag
                    nc.vector.tensor_tensor(sqr[:], sqr[:], sqi[:], op=ALU.add)
                    nc.scalar.activation(sqr[:], sqr[:], AF.Ln)
                    nc.scalar.activation(sqr[:], sqr[:], AF.Exp, scale=0.5)
                    et = pet.tile([128, OWN], bf16, tag="et",
                                  name=f"et{h}_{t2b}")
                    nc.scalar.activation(et[:], sqr[:], AF.Exp)
                    if debug and h == 0 and t2b == 0:
                        nc.sync.dma_start(dbg["d_et"][:], et[:])
                    et_fifo.append((h, t2b, et, vt_pair))
                    while len(et_fifo) > LAG:
                        emit_avden(et_fifo.pop(0))
            for ent in et_fifo:
                emit_avden(ent)
            et_fifo.clear()
            if debug:
                nc.sync.dma_start(dbg["d_attn"][:], attn[:])
            es_b.close()

            # ------------- phase D: proj + residual --------------------
            r1r = poolR1.tile([128, 12, OWN], bf16, name="r1r")
            with ExitStack() as es_d:
                pwp = es_d.enter_context(tc.tile_pool(name="pwp", bufs=3))
                ps_d = es_d.enter_context(
                    tc.tile_pool(name="ps_d", bufs=4, space="PSUM"))
                for opb in range(12):
                    rp_r = pwp.tile([64, 12, 128], bf16, tag="rpr",
                                    name=f"rpr{opb}")
                    rp_i = pwp.tile([64, 12, 128], bf16, tag="rpi",
                                    name=f"rpi{opb}")
                    j = opb % 6
                    nc.sync.dma_start(rp_r[:],
                                      w_proj[0, j].rearrange("k p n -> p k n"))
                    nc.sync.dma_start(rp_i[:],
                                      w_proj[1, j].rearrange("k p n -> p k n"))
                    wp_t = pwp.tile([128, 12, 128], bf16, tag="wp",
                                    name=f"wp{opb}")
                    if opb < 6:
                        nc.vector.tensor_scalar(wp_t[0:64, :, :], rp_r[:],
                                                1.0, None, op0=ALU.mult)
                        nc.vector.tensor_scalar(wp_t[64:128, :, :], rp_i[:],
                                                -1.0, None, op0=ALU.mult)
                    else:
                        nc.vector.tensor_scalar(wp_t[0:64, :, :], rp_i[:],
                                                1.0, None, op0=ALU.mult)
                        nc.vector.tensor_scalar(wp_t[64:128, :, :], rp_r[:],
                                                1.0, None, op0=ALU.mult)
                    wps_t = pwp.tile([4, 128], bf16, tag="wps",
                                     name=f"wps{opb}")
                    nc.sync.dma_start(wps_t[:], w_proj_s[opb])
                    pr_ps = ps_d.tile([128, OWN], f32, tag="pr",
                                      name=f"prps{opb}")
                    for kb in range(12):
                        nc.tensor.matmul(pr_ps[:], wp_t[:, kb, :],
                                         attn[:, kb, :],
                                         start=(kb == 0), stop=False)
                    nc.tensor.matmul(pr_ps[:], wps_t[:], t_stat_one[:],
                                     start=False, stop=True)
                    nc.vector.tensor_tensor(xr1[:, opb, :], pr_ps[:],
                                            xr1[:, opb, :], op=ALU.add)
                    nc.vector.tensor_copy(r1r[:, opb, :], xr1[:, opb, :])
            es_attn.close()

        # ---------------- phase E: LN2 stats over own tokens --------------
        stat2 = poolR1.tile([4, OWN], bf16, name="stat2")
        rstd2_bc = poolR1.tile([128, OWN], f32, name="rstd2_bc")
        nc.vector.memset(stat2[:], 0.0)
        with ExitStack() as es_e:
            pe = es_e.enter_context(tc.tile_pool(name="pe_sb", bufs=1))
            pdram2 = es_e.enter_context(
                tc.tile_pool(name="pdram2", bufs=1, space="DRAM"))
            pe_ps = es_e.enter_context(
                tc.tile_pool(name="pe_ps", bufs=2, space="PSUM"))
            sq2s = []
            for kb in range(12):
                sq2 = pe.tile([128, OWN], bf16, tag="sq2", name=f"sq2_{kb}", bufs=12)
                nc.scalar.activation(sq2[:], r1r[:, kb, :], AF.Square)
                sq2s.append(sq2)
            mu2_ps = pe_ps.tile([2, OWN], f32, tag="mu2", name="mu2")
            s2_ps = pe_ps.tile([1, OWN], f32, tag="s2", name="s2")
            for kb in range(12):
                lhs = t_ones_ab[:, 0:2] if kb < 6 else t_ones_ab[:, 2:4]
                nc.tensor.matmul(mu2_ps[:], lhs, r1r[:, kb, :],
                                 start=(kb == 0), stop=(kb == 11))
                nc.tensor.matmul(s2_ps[:], t_ones_s[:], sq2s[kb][:],
                                 start=(kb == 0), stop=(kb == 11))
            mu2_sb = pe.tile([2, OWN], f32, tag="emusb", name="emusb")
            mu2_fl = pe.tile([1, 2, OWN], f32, tag="emufl", name="emufl")
            var = pe.tile([1, OWN], f32, tag="evar", name="evar")
            lnv = pe.tile([1, OWN], f32, tag="elnv", name="elnv")
            nc.vector.tensor_copy(mu2_sb[:], mu2_ps[:])
            nc.sync.dma_start(mu2_fl[:, 0, :], mu2_sb[0:1, :])
            nc.sync.dma_start(mu2_fl[:, 1, :], mu2_sb[1:2, :])
            sq_mu2 = pe.tile([1, 2, OWN], f32, tag="esqmu", name="esqmu")
            nc.vector.tensor_tensor(sq_mu2[:], mu2_fl[:], mu2_fl[:], op=ALU.mult)
            nc.vector.tensor_tensor(var[:], s2_ps[:], sq_mu2[:, 0, :],
                                    op=ALU.subtract)
            nc.vector.tensor_tensor(var[:], var[:], sq_mu2[:, 1, :],
                                    op=ALU.subtract)
            nc.scalar.activation(lnv[:], var[:], AF.Ln, bias=t_eps[:])
            nc.vector.tensor_copy(stat2[0:2, :], mu2_sb[:])
            std2_row = pe.tile([1, OWN], bf16, tag="estd", name="estd")
            nc.scalar.activation(std2_row[:], lnv[:], AF.Exp, scale=0.5)
            nc.sync.dma_start(stat2[2:3, :], std2_row[:])
            rstd2_row = pe.tile([1, OWN], f32, tag="ers", name="ers")
            nc.scalar.activation(rstd2_row[:], lnv[:], AF.Exp, scale=-0.5)
            rstd2_dram = pdram2.tile([1, OWN], f32, name="rstd2_dram")
            nc.sync.dma_start(rstd2_dram[:], rstd2_row[:])
            bcast2 = bass.AP(tensor=rstd2_dram.tensor, offset=rstd2_dram[:].offset,
                             ap=[[0, 128]] + rstd2_dram[:].ap[1:])
            nc.sync.dma_start(rstd2_bc[:], bcast2)

        # ---------------- phase F: MLP per 256-token chunk ----------------
        with ExitStack() as es_f:
            pneg = es_f.enter_context(tc.tile_pool(name="pneg", bufs=1))
            r1neg = pneg.tile([128, 6, OWN], bf16, name="r1neg")
            for kb in range(6):
                nc.vector.tensor_scalar(r1neg[:, kb, :], r1r[:, 6 + kb, :],
                                        -1.0, None, op0=ALU.mult)
            ph = es_f.enter_context(tc.tile_pool(name="ph", bufs=1))
            phn = es_f.enter_context(tc.tile_pool(name="phn", bufs=1))
            pw1 = es_f.enter_context(tc.tile_pool(name="pw1", bufs=3))
            pw2 = es_f.enter_context(tc.tile_pool(name="pw2", bufs=8))
            pscf = es_f.enter_context(tc.tile_pool(name="pscf", bufs=4))
            pout = es_f.enter_context(tc.tile_pool(name="pout", bufs=2))
            ps_f = es_f.enter_context(
                tc.tile_pool(name="ps_f", bufs=6, space="PSUM"))
            for cc in range(OWN // MC):
                cs = slice(cc * MC, cc * MC + MC)
                h_t = ph.tile([128, 48, MC], bf16, tag="h", name=f"h{cc}")
                hn_t = phn.tile([128, 24, MC], bf16, tag="hn", name=f"hn{cc}")
                for Cb in range(24):
                    w1r_t = pw1.tile([128, 6, 128], bf16, tag="w1r",
                                     name=f"w1r{cc}_{Cb}")
                    w1i_t = pw1.tile([128, 6, 128], bf16, tag="w1i",
                                     name=f"w1i{cc}_{Cb}")
                    w1s_t = pw1.tile([4, 256], bf16, tag="w1s",
                                     name=f"w1s{cc}_{Cb}")
                    nc.sync.dma_start(w1r_t[:], w_fc1r[Cb]
                                      .rearrange("b p n -> p b n"))
                    nc.sync.dma_start(w1i_t[:], w_fc1i[Cb]
                                      .rearrange("b p n -> p b n"))
                    nc.sync.dma_start(w1s_t[:], w_fc1_s[Cb])
                    hr_ps = ps_f.tile([128, MC], f32, tag="fps",
                                      name=f"hrps{cc}_{Cb}")
                    hi_ps = ps_f.tile([128, MC], f32, tag="fps",
                                      name=f"hips{cc}_{Cb}")
                    for kb in range(6):
                        st = (kb == 0)
                        nc.tensor.matmul(hr_ps[:], w1r_t[:, kb, :],
                                         r1r[:, kb, cs], start=st, stop=False)
                        nc.tensor.matmul(hi_ps[:], w1i_t[:, kb, :],
                                         r1r[:, kb, cs], start=st, stop=False)
                    for kb in range(6):
                        nc.tensor.matmul(hr_ps[:], w1i_t[:, kb, :],
                                         r1neg[:, kb, cs], start=False, stop=False)
                        nc.tensor.matmul(hi_ps[:], w1r_t[:, kb, :],
                                         r1r[:, 6 + kb, cs], start=False,
                                         stop=False)
                    nc.tensor.matmul(hr_ps[:], w1s_t[:, 0:128], stat2[:, cs],
                                     start=False, stop=True)
                    nc.tensor.matmul(hi_ps[:], w1s_t[:, 128:256], stat2[:, cs],
                                     start=False, stop=True)
                    gr = pscf.tile([128, MC], f32, tag="g", name=f"gr{cc}_{Cb}")
                    gi = pscf.tile([128, MC], f32, tag="g", name=f"gi{cc}_{Cb}")
                    nc.vector.tensor_tensor(gr[:], hr_ps[:], rstd2_bc[:, cs],
                                            op=ALU.mult)
                    nc.vector.tensor_tensor(gi[:], hi_ps[:], rstd2_bc[:, cs],
                                            op=ALU.mult)
                    nc.scalar.activation(h_t[:, Cb, :], gr[:], AF.Gelu)
                    nc.scalar.activation(h_t[:, 24 + Cb, :], gi[:], AF.Gelu)
                    nc.vector.tensor_scalar(hn_t[:, Cb, :], h_t[:, 24 + Cb, :],
                                            -1.0, None, op0=ALU.mult)
                for j in range(6):
                    w2r_a = pw2.tile([128, 12, 128], bf16, tag="w2",
                                     name=f"w2ra{cc}_{j}")
                    w2r_b = pw2.tile([128, 12, 128], bf16, tag="w2",
                                     name=f"w2rb{cc}_{j}")
                    w2i_a = pw2.tile([128, 12, 128], bf16, tag="w2",
                                     name=f"w2ia{cc}_{j}")
                    w2i_b = pw2.tile([128, 12, 128], bf16, tag="w2",
                                     name=f"w2ib{cc}_{j}")
                    w2s_t = pw2.tile([4, 256], bf16, tag="w2s",
                                     name=f"w2s{cc}_{j}")
                    nc.sync.dma_start(w2r_a[:], w_fc2r[j, 0:12]
                                      .rearrange("b p n -> p b n"))
                    nc.sync.dma_start(w2r_b[:], w_fc2r[j, 12:24]
                                      .rearrange("b p n -> p b n"))
                    nc.sync.dma_start(w2i_a[:], w_fc2i[j, 0:12]
                                      .rearrange("b p n -> p b n"))
                    nc.sync.dma_start(w2i_b[:], w_fc2i[j, 12:24]
                                      .rearrange("b p n -> p b n"))
                    nc.sync.dma_start(w2s_t[:], w_fc2_s[j])
                    or_ps = ps_f.tile([128, MC], f32, tag="fps",
                                      name=f"orps{cc}_{j}")
                    oi_ps = ps_f.tile([128, MC], f32, tag="fps",
                                      name=f"oips{cc}_{j}")
                    for kb in range(24):
                        w2r = w2r_a[:, kb, :] if kb < 12 else w2r_b[:, kb - 12, :]
                        w2i = w2i_a[:, kb, :] if kb < 12 else w2i_b[:, kb - 12, :]
                        st = (kb == 0)
                        nc.tensor.matmul(or_ps[:], w2r, h_t[:, kb, :],
                                         start=st, stop=False)
                        nc.tensor.matmul(oi_ps[:], w2i, h_t[:, kb, :],
                                         start=st, stop=False)
                    for kb in range(24):
                        w2r = w2r_a[:, kb, :] if kb < 12 else w2r_b[:, kb - 12, :]
                        w2i = w2i_a[:, kb, :] if kb < 12 else w2i_b[:, kb - 12, :]
                        nc.tensor.matmul(or_ps[:], w2i, hn_t[:, kb, :],
                                         start=False, stop=False)
                        nc.tensor.matmul(oi_ps[:], w2r,
                                         h_t[:, 24 + kb, :],
                                         start=False, stop=False)
                    nc.tensor.matmul(or_ps[:], w2s_t[:, 0:128],
                                     t_stat_one[:, cs], start=False, stop=True)
                    nc.tensor.matmul(oi_ps[:], w2s_t[:, 128:256],
                                     t_stat_one[:, cs], start=False, stop=True)
                    o_r = pout.tile([128, MC], bf16, tag="o", name=f"or{cc}_{j}")
                    o_i = pout.tile([128, MC], bf16, tag="o", name=f"oi{cc}_{j}")
                    nc.vector.tensor_tensor(o_r[:], or_ps[:], xr1[:, j, cs],
                                            op=ALU.add)
                    nc.vector.tensor_tensor(o_i[:], oi_ps[:], xr1[:, 6 + j, cs],
                                            op=ALU.add)
                    nc.sync.dma_start(out_fm[j, :, cs], o_r[:])
                    nc.sync.dma_start(out_fm[6 + j, :, cs], o_i[:])
    nc.compile()
    return nc


# --------------------------------------------------------------------------
# host side
# --------------------------------------------------------------------------

def _cx(a):
    return a[..., 0].astype(np.float64) + 1j * a[..., 1].astype(np.float64)


def _prep_weights(inputs):
    n1 = _cx(inputs["n1_w"]); b1 = _cx(inputs["n1_b"])
    n2 = _cx(inputs["n2_w"]); b2 = _cx(inputs["n2_b"])
    Wqkv = _cx(inputs["qkv_w"])          # [2304, 768]
    Wp = _cx(inputs["proj_w"])           # [768, 768]
    bp = _cx(inputs["proj_b"])           # [768]
    W1 = _cx(inputs["fc1_w"])            # [3072, 768]
    bf1 = _cx(inputs["fc1_b"])           # [3072]
    W2 = _cx(inputs["fc2_w"])            # [768, 3072]
    bf2 = _cx(inputs["fc2_b"])           # [768]

    d = {}
    # ---- qkv (LN1-folded), raw transposed blocks [H, 6, 128, 64] ----
    Wq, Wk, Wv = Wqkv[0:768], Wqkv[768:1536], Wqkv[1536:2304]

    def fold1(W):
        Wf = W * n1[None, :]
        return Wf, Wf.sum(1), W @ b1

    w_att = np.zeros((6, H, 6, 128, 64), np.float32)
    w_qkv_s = np.zeros((H, 4, 384), np.float32)
    wvs = np.zeros((4, 1536), np.float32)
    for h in range(H):
        rows = slice(h * DH, (h + 1) * DH)
        Qf, Qs, Qb = fold1(Wq[rows])
        Kf, Ks, Kb_ = fold1(Wk[rows])
        Vf, Vs, Vb = fold1(Wv[rows])
        for pi, M in enumerate([Qf.real, Qf.imag, Kf.real, Kf.imag,
                                Vf.real, Vf.imag]):
            w_att[pi, h] = M.T.reshape(6, 128, 64)
        # stat/bias columns (rows mu_r, mu_i, std of the K-profile)
        # q1 stats: kcols-r x S | kcols-i x -S ; q3: i x S | r x S ; k: r | i
        def stat3(ws, wb, plane, scale=1.0):
            out = np.zeros((3, DH), np.float64)
            if plane == "r":
                out[0] = -ws.real
                out[1] = ws.imag
                out[2] = wb.real
            else:
                out[0] = -ws.imag
                out[1] = -ws.real
                out[2] = wb.imag
            return out * scale

        w_qkv_s[h, 0:3, 0:64] = stat3(Qs, Qb, "r", SCALE)
        w_qkv_s[h, 0:3, 64:128] = stat3(Qs, Qb, "i", -SCALE)
        w_qkv_s[h, 0:3, 128:192] = stat3(Qs, Qb, "i", SCALE)
        w_qkv_s[h, 0:3, 192:256] = stat3(Qs, Qb, "r", SCALE)
        w_qkv_s[h, 0:3, 256:320] = stat3(Ks, Kb_, "r")
        w_qkv_s[h, 0:3, 320:384] = stat3(Ks, Kb_, "i")
        base = h * 128
        wvs[0:3, base:base + 64] = stat3(Vs, Vb, "r")
        wvs[0:3, base + 64:base + 128] = stat3(Vs, Vb, "i")
    d["w_att"] = w_att
    d["w_qkv_s"] = w_qkv_s
    d["w_v_s"] = wvs

    # ---- proj raw transposed blocks [2, 6, 12, 64, 128] + bias ----
    w_proj = np.zeros((2, 6, 12, 64, 128), np.float32)
    w_proj_s = np.zeros((12, 4, 128), np.float32)
    for j in range(6):
        orow = slice(j * 128, (j + 1) * 128)
        for hh in range(H):
            cols = slice(hh * DH, (hh + 1) * DH)
            w_proj[0, j, hh] = Wp.real[orow, cols].T
            w_proj[1, j, hh] = Wp.imag[orow, cols].T
    for opb in range(12):
        orow = slice((opb % 6) * 128, (opb % 6) * 128 + 128)
        w_proj_s[opb, 0] = (bp.real if opb < 6 else bp.imag)[orow]
    d["w_proj"] = w_proj
    d["w_proj_s"] = w_proj_s

    # ---- fc1 (LN2-folded, shared-tile form) ----
    W1f = W1 * n2[None, :]
    W1s = W1f.sum(1)
    W1b = W1 @ b2 + bf1
    w_fc1r = np.zeros((24, 6, 128, 128), np.float32)
    w_fc1i = np.zeros((24, 6, 128, 128), np.float32)
    w_fc1_s = np.zeros((24, 4, 256), np.float32)
    for Cb in range(24):
        orow = slice(Cb * 128, (Cb + 1) * 128)
        for kb in range(6):
            icol = slice(kb * 128, (kb + 1) * 128)
            w_fc1r[Cb, kb] = W1f.real[orow, icol].T
            w_fc1i[Cb, kb] = W1f.imag[orow, icol].T
        w_fc1_s[Cb, 0, 0:128] = -W1s.real[orow]
        w_fc1_s[Cb, 1, 0:128] = W1s.imag[orow]
        w_fc1_s[Cb, 2, 0:128] = W1b.real[orow]
        w_fc1_s[Cb, 0, 128:256] = -W1s.imag[orow]
        w_fc1_s[Cb, 1, 128:256] = -W1s.real[orow]
        w_fc1_s[Cb, 2, 128:256] = W1b.imag[orow]
    d["w_fc1r"] = w_fc1r
    d["w_fc1i"] = w_fc1i
    d["w_fc1_s"] = w_fc1_s

    # ---- fc2 (plain + bias) ----
    w_fc2r = np.zeros((6, 24, 128, 128), np.float32)
    w_fc2i = np.zeros((6, 24, 128, 128), np.float32)
    w_fc2_s = np.zeros((6, 4, 256), np.float32)
    for j in range(6):
        orow = slice(j * 128, (j + 1) * 128)
        for kb in range(24):
            icol = slice(kb * 128, (kb + 1) * 128)
            w_fc2r[j, kb] = W2.real[orow, icol].T
            w_fc2i[j, kb] = W2.imag[orow, icol].T
        w_fc2_s[j, 0, 0:128] = bf2.real[orow]
        w_fc2_s[j, 0, 128:256] = bf2.imag[orow]
    d["w_fc2r"] = w_fc2r
    d["w_fc2i"] = w_fc2i
    d["w_fc2_s"] = w_fc2_s

    for k in list(d.keys()):
        d[k] = d[k].astype(BF16)

    # ---- consts ----
    d["ones_col"] = np.ones((128, 1), BF16)
    oab = np.zeros((128, 4), np.float32)
    oab[:, 0] = 1.0 / C
    oab[:, 3] = 1.0 / C
    d["ones_ab"] = oab.astype(BF16)
    d["ones_s"] = np.full((128, 1), 1.0 / C, np.float32).astype(BF16)
    so = np.zeros((4, OWN), np.float32)
    so[0] = 1.0
    d["stat_one"] = so.astype(BF16)
    return d


def _make_in_maps(wd, x):
    shard_arrs = {}
    for nm in W_SHAPES:
        flat = np.ascontiguousarray(wd[nm]).reshape(8, 128, _shard_cols(nm))
        shard_arrs[f"sh_{nm}"] = flat
    smalls = {k: v for k, v in wd.items() if k not in W_SHAPES}
    in_maps = []
    for c in range(NCORES):
        b, half = divmod(c, 2)
        xr_ = x[b, :, :, 0].T                        # [768, 1024]
        xi_ = x[b, :, :, 1].T
        stack = np.concatenate([xr_, xi_], 0)        # [1536, 1024]
        own = stack[:, half * OWN:(half + 1) * OWN]
        m = dict(smalls)
        for nm in W_SHAPES:
            m[f"sh_{nm}"] = shard_arrs[f"sh_{nm}"][c]
        m["x_own"] = np.ascontiguousarray(own).astype(BF16).reshape(12, 128, OWN)
        in_maps.append(m)
    return in_maps


_NC_CACHE = {}


def kernel(**inputs):
    if "nc" not in _NC_CACHE:
        _NC_CACHE["nc"] = build_nc()
    nc = _NC_CACHE["nc"]

    wd = _prep_weights(inputs)
    x = np.asarray(inputs["x"], np.float32)          # [B, N, C, 2]
    in_maps = _make_in_maps(wd, x)

    res = run_bass_kernel_spmd(nc, in_maps, list(range(NCORES)))
    out = np.empty((B, N, C, 2), np.float32)
    for c in range(NCORES):
        b, half = divmod(c, 2)
        o = np.asarray(res.results[c]["out_fm"], dtype=np.float32)
        sl = slice(half * OWN, half * OWN + OWN)
        out[b, sl, :, 0] = o[0:6].reshape(768, OWN).T
        out[b, sl, :, 1] = o[6:12].reshape(768, OWN).T
    return out
